# revision 1
# baseline (speedup 1.0000x reference)
"""Trainium2 Bass kernel for the RGCN message-passing model (nn_Actor_12094627905962).

Strategy (8 NeuronCores, dst-sharded):
  - Each core owns a contiguous range of NS=12500 destination nodes and all
    edges pointing into them.
  - Layer 1 is fully gather-free: for every (pass, chunk) slot the host packs
    a relation-masked input column U[s] = concat_r[mask_r*x_aug(src) ;
    mask_r*ea_aug(e)] (56 rows).  On device md = relu(Wblk^T U) (768-dim via 6
    matmuls) and msg = Wstack^T md accumulates straight into the PSUM tile of
    the destination chunk (relu(0)=0 makes dummy slots free).  This replaces
    the n-table, its AllGather, the e-term table and all L1 indirect gathers.
  - Layer 2 (max aggregation): per-relation transformed tables of the layer-1
    output are AllGathered, then multi-pass [128,1] indirect gathers fill
    per-(dst,rel) slot accumulators with DVE max folds (as before, minus the
    per-chunk index copies).
  - stage2 (sum of per-relation maxes) and the dense head are unchanged.

All float math runs on device; the host side only shards/permutes inputs and
builds the masked U matrix + u32 gather index tables.
"""

import sys

if "/opt/trn_rl_repo" not in sys.path:
    sys.path.insert(0, "/opt/trn_rl_repo")

import numpy as np
import ml_dtypes

BF = ml_dtypes.bfloat16

N = 100_000
E = 1_600_000
R = 8
NCORES = 8
NS = N // NCORES  # 12500
D = 64
NSP = 12800  # node positions padded (100 chunks of 128)
NCH = NSP // 128  # 100
SEG = R * NSP + 2  # rows per rank segment in gathered H table (+zero,+neg rows)
ZROW = R * NSP
NROW = R * NSP + 1
NEGBIG = -1.0e30
L2_SLAB_CHUNKS = 234


def _ceil(a, b):
    return -(-a // b)


# ---------------------------------------------------------------------------
# host-side preprocessing
# ---------------------------------------------------------------------------


def preprocess(edge_index, edge_type):
    """Pure index preprocessing. Returns per-core structures + common sizes."""
    src = np.asarray(edge_index[0], np.int64)
    dst = np.asarray(edge_index[1], np.int64)
    rel = np.asarray(edge_type, np.int64)

    core_of_edge = dst // NS
    cores = []
    for c in range(NCORES):
        m = np.nonzero(core_of_edge == c)[0]
        cores.append({"eids": m, "s": src[m], "d": dst[m] - c * NS, "r": rel[m]})

    # global rank: per core, nodes ordered by layer-1 in-degree (desc)
    grank = np.empty(N, np.int64)
    for c in range(NCORES):
        deg = np.bincount(cores[c]["d"], minlength=NS)
        order = np.argsort(-deg, kind="stable")
        rank = np.empty(NS, np.int64)
        rank[order] = np.arange(NS)
        cores[c]["deg"] = deg
        cores[c]["rank"] = rank
        grank[c * NS : (c + 1) * NS] = rank

    for c in range(NCORES):
        cc = cores[c]
        s, d, r = cc["s"], cc["d"], cc["r"]
        ne = len(s)
        rank = cc["rank"]

        # --- L1: per-node edge slot (j-th edge of its dst) ---
        dorder = np.argsort(d, kind="stable")
        ds = d[dorder]
        starts = np.searchsorted(ds, np.arange(NS))
        j1 = np.arange(ne) - starts[ds]
        cc["l1_edge"] = dorder
        cc["l1_j"] = j1
        cc["l1_pos"] = rank[ds]
        cc["maxd1"] = int(cc["deg"].max()) if ne else 0
        degsorted = -np.sort(-cc["deg"])
        cc["cnt1"] = np.array(
            [int((degsorted >= j + 1).sum()) for j in range(cc["maxd1"])], np.int64
        )

        # --- L2: (dst, rel) groups ---
        g = d * R + r
        gorder = np.argsort(g, kind="stable")
        gs = g[gorder]
        uniq, uidx, ucnt = np.unique(gs, return_index=True, return_counts=True)
        ngroups = len(uniq)
        grank2 = np.argsort(-ucnt, kind="stable")
        slot_of_u = np.empty(ngroups, np.int64)
        slot_of_u[grank2] = np.arange(ngroups)
        gid_of_edge = np.searchsorted(uniq, gs)
        j2 = np.arange(ne) - uidx[gid_of_edge]
        cc["l2_edge"] = gorder
        cc["l2_j"] = j2
        cc["l2_slot"] = slot_of_u[gid_of_edge]
        cc["l2_cnt"] = ucnt[gid_of_edge]  # group size per sorted edge
        cc["l2_ngroups"] = ngroups
        cc["l2_n2"] = int((ucnt >= 2).sum())  # slotted (non-singleton) groups
        cc["l2_ucnt"] = ucnt
        cc["l2_uidx"] = uidx
        cc["maxd2"] = int(ucnt.max()) if ne else 0
        csorted = -np.sort(-ucnt)
        cc["cnt2"] = np.array(
            [int((csorted >= j + 1).sum()) for j in range(cc["maxd2"])], np.int64
        )
        cc["s2_dst"] = uniq // R
        cc["s2_slot"] = slot_of_u

    # ---- common (max-over-cores) sizes ----
    maxd1 = max(c["maxd1"] for c in cores)
    maxd2 = max(c["maxd2"] for c in cores)
    cmax1 = np.zeros(maxd1, np.int64)
    cmax2 = np.zeros(maxd2, np.int64)
    for c in cores:
        cmax1[: c["maxd1"]] = np.maximum(cmax1[: c["maxd1"]], c["cnt1"])
        cmax2[: c["maxd2"]] = np.maximum(cmax2[: c["maxd2"]], c["cnt2"])
    C1 = np.array([_ceil(int(x), 128) for x in cmax1], np.int64)
    SLOTMAX = max(c["l2_n2"] for c in cores)  # only count>=2 groups get slots
    CH2 = _ceil(SLOTMAX, 128)
    C2 = np.array([_ceil(int(x), 128) for x in cmax2], np.int64)
    C2[0] = CH2  # pass 0 (bypass) covers all slotted groups incl. dummy slots

    # stage2 chunk extents: furthest rank position with > k nonempty rels
    CS2 = np.zeros(8, np.int64)
    for c in range(NCORES):
        cc = cores[c]
        s2d = cc["s2_dst"]
        rank = cc["rank"]
        dorder2 = np.argsort(s2d, kind="stable")
        sd = s2d[dorder2]
        st = np.searchsorted(sd, np.arange(NS))
        en = np.searchsorted(sd, np.arange(NS), side="right")
        nrels = en - st
        cc["s2_dorder"] = dorder2
        cc["s2_st"] = st
        cc["s2_nrels"] = nrels
        for k in range(8):
            m = np.nonzero(nrels > k)[0]
            if len(m):
                CS2[k] = max(CS2[k], _ceil(int(rank[m].max()) + 1, 128))
    ncol_l1 = int(C1.sum())
    ncol_l2 = int(C2.sum())
    col_l2 = np.concatenate([[0], np.cumsum(C2)])[:-1]
    col_s2 = ncol_l2 + np.concatenate([[0], np.cumsum(CS2)])[:-1]
    TC = ncol_l2 + int(CS2.sum())
    ucol = np.concatenate([[0], np.cumsum(C1)])[:-1]  # U column-chunk base per pass

    common = {
        "maxd1": maxd1,
        "maxd2": maxd2,
        "C1": C1,
        "C2": C2,
        "CS2": CS2,
        "CH2": CH2,
        "SLOTMAX": SLOTMAX,
        "grank": grank,
        "TC": TC,
        "col_l2": col_l2,
        "col_s2": col_s2,
        "ucol": ucol,
        "ncol_l1": ncol_l1,
        "SL1": ncol_l1 * 128,
    }

    # ---- per-core index arenas (L2 passes + stage2 only) ----
    for c in range(NCORES):
        cc = cores[c]
        s, d, r = cc["s"], cc["d"], cc["r"]
        arena = np.zeros((128, TC), np.uint32)

        def _htab_row(rr, ss):
            return (ss // NS) * SEG + rr * NSP + grank[ss]

        ndummy = np.uint32(NROW)  # core0 segment -big row

        def _fill(colbase, nchunks, positions, rows, dummy):
            block = np.full(nchunks * 128, dummy, np.uint32)
            block[positions] = rows.astype(np.uint32)
            arena[:, colbase : colbase + nchunks] = block.reshape(nchunks, 128).T

        # L2 passes (singleton groups are skipped; stage2 reads them directly)
        ge, gj, gslot, gcnt = cc["l2_edge"], cc["l2_j"], cc["l2_slot"], cc["l2_cnt"]
        for j in range(maxd2):
            m = (gj == j) & (gcnt >= 2)
            nchunk = int(C2[j])
            pos = gslot[m]
            eids = ge[m]
            hrows = _htab_row(r[eids], s[eids])
            _fill(int(col_l2[j]), nchunk, pos, hrows, ndummy)

        # stage2 passes: node at position p gets its k-th group's value row --
        # slot row (count>=2, in the s2buf region at QOFF) or the single
        # edge's H-table row (singleton groups)
        QOFF = NCORES * SEG
        s2slot = cc["s2_slot"]
        ucnt2 = cc["l2_ucnt"]
        uidx2 = cc["l2_uidx"]
        ge2 = cc["l2_edge"]
        rank = cc["rank"]
        dorder2 = cc["s2_dorder"]
        st = cc["s2_st"]
        nrels = cc["s2_nrels"]
        for k in range(8):
            m = nrels > k
            pos = rank[np.nonzero(m)[0]]
            uids = dorder2[st[m] + k]
            first_e = ge2[uidx2[uids]]  # first edge of each group
            single_rows = _htab_row(r[first_e], s[first_e])
            rows = np.where(
                ucnt2[uids] >= 2, QOFF + s2slot[uids], single_rows
            ).astype(np.uint32)
            _fill(int(col_s2[k]), int(CS2[k]), pos, rows, np.uint32(QOFF + CH2 * 128))

        cc["arena"] = arena

        # --- U column index per edge (L1 masked input) ---
        # sorted-edge t: edge id l1_edge[t], slot = (ucol[j]+0)*128 + l1_pos[t]
        cc["uslot"] = ucol[cc["l1_j"]] * 128 + cc["l1_pos"]

    return cores, common


def build_core_inputs(inputs, cores, common):
    """Per-core numpy input dict."""
    x = np.asarray(inputs["x"], np.float32)
    ea = np.asarray(inputs["edge_attr"], np.float32)
    om = np.asarray(inputs["omega"], np.float32)

    f = lambda k: np.asarray(inputs[k], np.float32)
    Wn, bn = f("Wn"), f("bn")
    We, be = f("We"), f("be")
    Wo, bo = f("Wo"), f("bo")
    W1, Wroot1, b1 = f("W1"), f("Wroot1"), f("b1")
    W2, Wroot2, b2 = f("W2"), f("Wroot2"), f("b2")
    Wagg, bagg = f("Wagg"), f("bagg")
    Wc, bc = f("Wc"), f("bc")

    # f32 weight pack [10, 64]: Wnx 0:4, Wox 4:7, Wex (unused rows kept for layout)
    wf = np.zeros((10, 64), np.float32)
    wf[0:3, :] = Wn
    wf[3, :] = bn
    wf[4:6, :] = Wo
    wf[6, :] = bo

    # bf16 pack rows (same offsets as before where still used):
    #   0:512   Wstack n-part (64r+i -> W1[r, i, :])
    #   512:768 Wstack e-part (512+32r+i -> W1[r, 64+i, :])
    #   768:1280 W2
    #   1280:1345 Wroot1 + b1
    #   1345:1410 Wroot2 + b2
    #   1410:1475 WaggA + bagg
    #   1475:1539 WaggB
    #   1539:1604 Wc + bc (col 0)
    wb = np.zeros((1604, 64), np.float32)
    wb[0:512] = W1[:, :64, :].reshape(512, 64)
    wb[512:768] = W1[:, 64:96, :].reshape(256, 64)
    wb[768:1280] = W2.reshape(512, 64)
    wb[1280:1344] = Wroot1
    wb[1344] = b1
    wb[1345:1409] = Wroot2
    wb[1409] = b2
    wb[1410:1474] = Wagg[:64]
    wb[1474] = bagg
    wb[1475:1539] = Wagg[64:]
    wb[1539:1603, 0] = Wc[:, 0]
    wb[1603, 0] = bc[0]
    wb = wb.astype(BF)

    # block-diagonal masked-encoder weights [56, 768]
    Wn_aug = np.vstack([Wn, bn[None, :]])  # [4, 64]
    We_aug = np.vstack([We, be[None, :]])  # [3, 32]
    wq = np.zeros((56, 768), np.float32)
    for r in range(R):
        wq[4 * r : 4 * r + 4, 64 * r : 64 * r + 64] = Wn_aug
        wq[32 + 3 * r : 32 + 3 * r + 3, 512 + 32 * r : 512 + 32 * r + 32] = We_aug
    wq = wq.astype(BF)

    SL1 = common["SL1"]
    in_maps = []
    for c in range(NCORES):
        cc = cores[c]
        rank = cc["rank"]
        inv = np.argsort(rank)  # position -> node
        xT = np.zeros((4, NSP), np.float32)
        xT[:3, :NS] = x[c * NS : (c + 1) * NS][inv].T
        xT[3, :] = 1.0
        omT = np.zeros((3, NSP), np.float32)
        omT[:2, :NS] = om[c * NS : (c + 1) * NS][inv].T
        omT[2, :] = 1.0

        # masked U matrix [56, SL1]
        U = np.zeros((56, SL1), np.float32)
        eids = cc["eids"][cc["l1_edge"]]  # global edge id per sorted edge
        ssrc = cc["s"][cc["l1_edge"]]
        rr = cc["r"][cc["l1_edge"]]
        sl = cc["uslot"]
        U[4 * rr + 0, sl] = x[ssrc, 0]
        U[4 * rr + 1, sl] = x[ssrc, 1]
        U[4 * rr + 2, sl] = x[ssrc, 2]
        U[4 * rr + 3, sl] = 1.0
        U[32 + 3 * rr + 0, sl] = ea[eids, 0]
        U[32 + 3 * rr + 1, sl] = ea[eids, 1]
        U[32 + 3 * rr + 2, sl] = 1.0

        in_maps.append(
            {
                "xT": xT,
                "omT": omT,
                "wf": wf,
                "wb": wb,
                "wq": wq,
                "U": U.astype(BF),
                "idxs": cc["arena"],
            }
        )
    return in_maps


# ---------------------------------------------------------------------------
# device graph
# ---------------------------------------------------------------------------


SKIP_L1MD = False
SKIP_L2 = False
SKIP_S2 = False


def build_graph(common):
    import concourse.bacc as bacc
    import concourse.bass as bass
    import concourse.mybir as mybir
    from concourse.tile import TileContext
    from concourse.masks import make_identity

    fp32 = mybir.dt.float32
    bf16 = mybir.dt.bfloat16
    u32 = mybir.dt.uint32
    AX = mybir.AluOpType

    C1, C2 = common["C1"], common["C2"]
    CS2 = common["CS2"]
    maxd1, maxd2 = common["maxd1"], common["maxd2"]
    col_l2, col_s2 = common["col_l2"], common["col_s2"]
    ucol = common["ucol"]
    CH2 = common["CH2"]
    TC = common["TC"]
    SL1 = common["SL1"]
    S2ROWS = CH2 * 128 + 1

    nc = bacc.Bacc(None, target_bir_lowering=False, num_swdge_queues=4)

    xT = nc.dram_tensor("xT", [4, NSP], fp32, kind="ExternalInput")
    omT = nc.dram_tensor("omT", [3, NSP], fp32, kind="ExternalInput")
    wf = nc.dram_tensor("wf", [10, 64], fp32, kind="ExternalInput")
    wb = nc.dram_tensor("wb", [1604, 64], bf16, kind="ExternalInput")
    wq_d = nc.dram_tensor("wq", [56, 768], bf16, kind="ExternalInput")
    U_d = nc.dram_tensor("U", [56, SL1], bf16, kind="ExternalInput")
    idxs_d = nc.dram_tensor("idxs", [128, TC], u32, kind="ExternalInput")
    out_d = nc.dram_tensor("out", [128, NCH], fp32, kind="ExternalOutput")

    hT_in = nc.dram_tensor("hT_in", [64, NSP], bf16)
    hT_all = nc.dram_tensor("hT_all", [NCORES * 64, NSP], bf16, addr_space="Shared")
    QOFF = NCORES * SEG  # s2buf region lives after the H-table in one tensor
    agH_out = nc.dram_tensor("agH_out", [QOFF + S2ROWS, 64], bf16)

    groups = [list(range(NCORES))]

    # passes per chunk for L1
    npass_of_chunk = [sum(1 for j in range(maxd1) if C1[j] > ch) for ch in range(NCH)]

    with TileContext(nc) as tc:
        with (
            tc.tile_pool(name="persist", bufs=1) as pp,
            tc.tile_pool(name="work", bufs=3) as wk,
        ):
            # ---- persistent tiles ----
            idxs = pp.tile([128, TC], u32)
            nc.sync.dma_start(out=idxs[:], in_=idxs_d[:, :])

            wnx = pp.tile([4, 64], fp32)
            nc.sync.dma_start(out=wnx[:], in_=wf[0:4, :])
            wox = pp.tile([3, 64], fp32)
            nc.sync.dma_start(out=wox[:], in_=wf[4:7, :])
            wqt = pp.tile([56, 768], bf16)
            nc.sync.dma_start(out=wqt[:], in_=wq_d[:, :])
            wstk = pp.tile([128, 6, 64], bf16)
            for k in range(6):
                nc.sync.dma_start(
                    out=wstk[:, k, :], in_=wb[128 * k : 128 * (k + 1), :]
                )
            w2a = pp.tile([64, R * 64], bf16)
            for rr in range(R):
                nc.sync.dma_start(
                    out=w2a[:, rr * 64 : (rr + 1) * 64],
                    in_=wb[768 + rr * 64 : 768 + (rr + 1) * 64, :],
                )
            wroot1 = pp.tile([65, 64], bf16)
            nc.sync.dma_start(out=wroot1[:], in_=wb[1280:1345, :])
            wroot2 = pp.tile([65, 64], bf16)
            nc.sync.dma_start(out=wroot2[:], in_=wb[1345:1410, :])
            wagga = pp.tile([65, 64], bf16)
            nc.sync.dma_start(out=wagga[:], in_=wb[1410:1475, :])
            waggb = pp.tile([64, 64], bf16)
            nc.sync.dma_start(out=waggb[:], in_=wb[1475:1539, :])
            wcx = pp.tile([65, 64], bf16)
            nc.sync.dma_start(out=wcx[:], in_=wb[1539:1604, :])

            ident = pp.tile([128, 128], bf16)
            make_identity(nc, ident[:])

            zrow = pp.tile([1, 64], bf16)
            nc.vector.memset(zrow[:], 0.0)
            nrow = pp.tile([1, 64], bf16)
            nc.vector.memset(nrow[:], NEGBIG)

            # ---- own-shard encoder nT (feature-major) ----
            nT = pp.tile([65, NSP], bf16)
            nc.vector.memset(nT[64:65, :], 1.0)
            with tc.tile_pool(name="psA", bufs=2, space="PSUM") as psA:
                for b in range(NSP // 512):
                    sl = slice(b * 512, (b + 1) * 512)
                    xch = wk.tile([4, 512], fp32, tag="xch")
                    nc.sync.dma_start(out=xch[:], in_=xT[:, sl])
                    p1 = psA.tile([64, 512], fp32, space="PSUM", tag="pa")
                    nc.tensor.matmul(p1[:], lhsT=wnx[:], rhs=xch[:], start=True, stop=True)
                    nc.scalar.activation(
                        nT[0:64, sl], p1[:], mybir.ActivationFunctionType.Relu
                    )

            # ---- layer 1: masked dense compute (j-outer, SBUF accumulator) ----
            hT = pp.tile([65, NSP], bf16)
            nc.vector.memset(hT[64:65, :], 1.0)
            UBLK = 32  # chunks of U per staged load
            with tc.tile_pool(name="psMd", bufs=2, space="PSUM") as psMd, tc.tile_pool(
                name="psAcc", bufs=2, space="PSUM"
            ) as psAcc, tc.tile_pool(
                name="psT", bufs=2, space="PSUM"
            ) as psT, tc.tile_pool(name="uw", bufs=3) as uw, tc.tile_pool(
                name="uv", bufs=8
            ) as uv, tc.tile_pool(name="a1p", bufs=1) as a1p:
                acc1 = a1p.tile([128, NCH, 64], fp32)
                nc.vector.memset(acc1[:], 0.0)
                for jj in range(0 if SKIP_L1MD else maxd1):
                    nchj = int(C1[jj])
                    for b0 in range(0, nchj, UBLK):
                        b1 = min(nchj, b0 + UBLK)
                        ucb = (int(ucol[jj]) + b0) * 128
                        ust = uw.tile([56, UBLK * 128], bf16, tag="ust")
                        nc.sync.dma_start(
                            out=ust[:, 0 : (b1 - b0) * 128],
                            in_=U_d[:, ucb : ucb + (b1 - b0) * 128],
                        )
                        for ch in range(b0, b1):
                            us = slice((ch - b0) * 128, (ch - b0 + 1) * 128)
                            mdp1 = psMd.tile([128, 384], fp32, space="PSUM", tag="md1")
                            mdp2 = psMd.tile([128, 384], fp32, space="PSUM", tag="md2")
                            for k in range(6):
                                tgt = mdp1 if k < 3 else mdp2
                                fs = slice((k % 3) * 128, (k % 3) * 128 + 128)
                                nc.tensor.matmul(
                                    tgt[:, fs],
                                    lhsT=wqt[:, 128 * k : 128 * (k + 1)],
                                    rhs=ust[:, us],
                                    start=True,
                                    stop=True,
                                )
                            mds = uv.tile([128, 6, 128], bf16, tag="mds")
                            nc.scalar.activation(
                                mds[:, 0:3, :].rearrange("p a f -> p (a f)"),
                                mdp1[:],
                                mybir.ActivationFunctionType.Relu,
                            )
                            nc.scalar.activation(
                                mds[:, 3:6, :].rearrange("p a f -> p (a f)"),
                                mdp2[:],
                                mybir.ActivationFunctionType.Relu,
                            )
                            msg = psAcc.tile([128, 64], fp32, space="PSUM", tag="msg")
                            for k in range(6):
                                nc.tensor.matmul(
                                    msg[:],
                                    lhsT=mds[:, k, :],
                                    rhs=wstk[:, k, :],
                                    start=(k == 0),
                                    stop=(k == 5),
                                )
                            nc.vector.tensor_tensor(
                                acc1[:, ch, :], acc1[:, ch, :], msg[:], op=AX.add
                            )
                # finalize: root term + accumulated messages, relu, transpose
                for ch in range(NCH):
                    sl = slice(ch * 128, (ch + 1) * 128)
                    p = psAcc.tile([128, 64], fp32, space="PSUM", tag="msg")
                    nc.tensor.matmul(
                        p[:], lhsT=nT[:, sl], rhs=wroot1[:], start=True, stop=True
                    )
                    nc.vector.tensor_tensor(p[:], p[:], acc1[:, ch, :], op=AX.add)
                    hch = wk.tile([128, 64], bf16, tag="hch")
                    nc.vector.tensor_scalar_max(hch[:], p[:], 0.0)
                    pt = psT.tile([64, 128], bf16, space="PSUM", tag="pt")
                    nc.tensor.transpose(pt[:], hch[:], ident[:])
                    nc.scalar.activation(
                        hT[0:64, sl], pt[:], mybir.ActivationFunctionType.Copy
                    )

            # ---- AllGather raw hT, then build the 8-relation table locally ----
            nc.sync.dma_start(out=hT_in[:, :], in_=hT[0:64, :])
            nc.gpsimd.collective_compute(
                "AllGather",
                mybir.AluOpType.bypass,
                replica_groups=groups,
                ins=[hT_in[:, :]],
                outs=[hT_all[:, :]],
            )
            with tc.tile_pool(name="psB", bufs=2, space="PSUM") as psB, tc.tile_pool(
                name="hw", bufs=3
            ) as hw:
                for c in range(NCORES):
                    for b in range(NSP // 512):
                        hg = hw.tile([64, 512], bf16, tag="hg")
                        nc.sync.dma_start(
                            out=hg[:],
                            in_=hT_all[c * 64 : (c + 1) * 64, b * 512 : (b + 1) * 512],
                        )
                        for q in range(4):
                            ch = b * 4 + q
                            p = psB.tile([128, R * 64], fp32, space="PSUM", tag="p")
                            nc.tensor.matmul(
                                p[:],
                                lhsT=hg[:, q * 128 : (q + 1) * 128],
                                rhs=w2a[:],
                                start=True,
                                stop=True,
                            )
                            stagH = wk.tile([128, R, 64], bf16, tag="stagH")
                            nc.scalar.activation(
                                stagH[:].rearrange("p r f -> p (r f)"),
                                p[:],
                                mybir.ActivationFunctionType.Copy,
                            )
                            nc.sync.dma_start(
                                out=agH_out[c * SEG : c * SEG + R * NSP, :]
                                .rearrange("(r ch p) f -> ch p r f", p=128, ch=NCH)[ch],
                                in_=stagH[:],
                            )
            nc.sync.dma_start(out=agH_out[ZROW : ZROW + 1, :], in_=zrow[:])
            nc.sync.dma_start(out=agH_out[NROW : NROW + 1, :], in_=nrow[:])

            # ---- layer-2 max: per-chunk staged gathers + DVE max folds (slabs) ----
            L2SLAB = L2_SLAB_CHUNKS
            nslabs = _ceil(CH2, L2SLAB)
            with tc.tile_pool(name="l2p", bufs=2) as l2p, tc.tile_pool(
                name="gp2", bufs=32
            ) as gp2:
                for sb in range(nslabs):
                    sch0 = sb * L2SLAB
                    sch1 = min(CH2, sch0 + L2SLAB)
                    a2s = l2p.tile([128, L2SLAB, 64], bf16, tag="a2s")
                    for j in range(0 if SKIP_L2 else maxd2):
                        c0 = int(col_l2[j])
                        hi = min(int(C2[j]), sch1)
                        for ch in range(sch0, hi):
                            if j == 0:
                                # pass 0 covers every slot: gather in place,
                                # no memset / staging / max-fold needed
                                gih = nc.gpsimd.indirect_dma_start(
                                    out=a2s[:, ch - sch0, :],
                                    out_offset=None,
                                    in_=agH_out[:, :],
                                    in_offset=bass.IndirectOffsetOnAxis(
                                        ap=idxs[:, c0 + ch : c0 + ch + 1], axis=0
                                    ),
                                )
                                gih.ins.queue = f"qPoolDynamic{ch % 4 or ''}"
                                continue
                            stg = gp2.tile([128, 64], bf16, tag="stg")
                            gih = nc.gpsimd.indirect_dma_start(
                                out=stg[:, :],
                                out_offset=None,
                                in_=agH_out[:, :],
                                in_offset=bass.IndirectOffsetOnAxis(
                                    ap=idxs[:, c0 + ch : c0 + ch + 1], axis=0
                                ),
                            )
                            gih.ins.queue = f"qPoolDynamic{ch % 4 or ''}" 
                            nc.vector.tensor_tensor(
                                a2s[:, ch - sch0, :],
                                a2s[:, ch - sch0, :],
                                stg[:],
                                op=AX.max,
                            )
                    nc.sync.dma_start(
                        out=agH_out[QOFF + sch0 * 128 : QOFF + sch1 * 128, :].rearrange(
                            "(ch p) f -> p ch f", p=128
                        ),
                        in_=a2s[:, 0 : sch1 - sch0, :],
                    )
            nc.sync.dma_start(
                out=agH_out[QOFF + CH2 * 128 : QOFF + CH2 * 128 + 1, :], in_=zrow[:]
            )

            # ---- stage2: per-node sum of its nonempty-rel maxes ----
            acc2e = pp.tile([128, NCH, 64], bf16)
            nc.vector.memset(acc2e[:], 0.0)
            with tc.tile_pool(name="gp3", bufs=32) as gp3:
                for ch in range(0 if SKIP_S2 else int(CS2.max())):
                    for k in range(8):
                        if ch >= int(CS2[k]):
                            continue
                        c0 = int(col_s2[k])
                        if k == 0:
                            # k=0 covers every node (zero row for edgeless
                            # ones): gather in place over the memset
                            gih = nc.gpsimd.indirect_dma_start(
                                out=acc2e[:, ch, :],
                                out_offset=None,
                                in_=agH_out[:, :],
                                in_offset=bass.IndirectOffsetOnAxis(
                                    ap=idxs[:, c0 + ch : c0 + ch + 1], axis=0
                                ),
                            )
                            gih.ins.queue = f"qPoolDynamic{(ch + k) % 4 or ''}"
                            continue
                        stg = gp3.tile([128, 64], bf16, tag="stg")
                        gih = nc.gpsimd.indirect_dma_start(
                            out=stg[:, :],
                            out_offset=None,
                            in_=agH_out[:, :],
                            in_offset=bass.IndirectOffsetOnAxis(
                                ap=idxs[:, c0 + ch : c0 + ch + 1], axis=0
                            ),
                        )
                        gih.ins.queue = f"qPoolDynamic{(ch + k) % 4 or ''}"
                        nc.vector.tensor_tensor(
                            acc2e[:, ch, :], acc2e[:, ch, :], stg[:], op=AX.add
                        )

            # ---- h2 = relu(root2 + acc2e); head ----
            y = pp.tile([128, NCH], fp32)
            psC_cm = tc.tile_pool(name="psC", bufs=4, space="PSUM")
            psC = psC_cm.__enter__()
            for ch in range(NCH):
                sl = slice(ch * 128, (ch + 1) * 128)
                p = psC.tile([128, 64], fp32, space="PSUM", tag="pc")
                nc.tensor.matmul(
                    p[:], lhsT=hT[:, sl], rhs=wroot2[:], start=True, stop=True
                )
                nc.vector.tensor_tensor(p[:], p[:], acc2e[:, ch, :], op=AX.add)
                h2 = wk.tile([128, 64], bf16, tag="h2")
                nc.vector.tensor_scalar_max(h2[:], p[:], 0.0)
                pt = psC.tile([64, 128], bf16, space="PSUM", tag="pc")
                nc.tensor.transpose(pt[:], h2[:], ident[:])
                h2T = wk.tile([65, 128], bf16, tag="h2T")
                nc.scalar.activation(
                    h2T[0:64, :], pt[:], mybir.ActivationFunctionType.Copy
                )
                nc.vector.memset(h2T[64:65, :], 1.0)
                if ch % 4 == 0:
                    sl4 = slice(ch * 128, min((ch + 4) * 128, NSP))
                    omch = wk.tile([3, 512], fp32, tag="omch")
                    nc.sync.dma_start(out=omch[:, 0 : sl4.stop - sl4.start], in_=omT[:, sl4])
                    po4 = psC.tile([64, 512], fp32, space="PSUM", tag="po4")
                    nc.tensor.matmul(
                        po4[:], lhsT=wox[:], rhs=omch[:], start=True, stop=True
                    )
                    oT4 = wk.tile([64, 512], bf16, tag="oT4")
                    nc.scalar.activation(
                        oT4[:], po4[:], mybir.ActivationFunctionType.Relu
                    )
                oTc = oT4[:, (ch % 4) * 128 : (ch % 4 + 1) * 128]
                p3 = psC.tile([128, 64], fp32, space="PSUM", tag="pc")
                nc.tensor.matmul(p3[:], lhsT=h2T[:], rhs=wagga[:], start=True, stop=False)
                nc.tensor.matmul(
                    p3[:], lhsT=oTc, rhs=waggb[:], start=False, stop=True
                )
                h3 = wk.tile([128, 64], bf16, tag="h3")
                nc.vector.tensor_scalar_max(h3[:], p3[:], 0.0)
                pt2 = psC.tile([64, 128], bf16, space="PSUM", tag="pc")
                nc.tensor.transpose(pt2[:], h3[:], ident[:])
                h3T = wk.tile([65, 128], bf16, tag="h3T")
                nc.scalar.activation(
                    h3T[0:64, :], pt2[:], mybir.ActivationFunctionType.Copy
                )
                nc.vector.memset(h3T[64:65, :], 1.0)
                py = psC.tile([128, 64], fp32, space="PSUM", tag="pc")
                nc.tensor.matmul(py[:], lhsT=h3T[:], rhs=wcx[:], start=True, stop=True)
                nc.scalar.activation(
                    y[:, ch : ch + 1],
                    py[:, 0:1],
                    mybir.ActivationFunctionType.Tanh,
                )
            nc.vector.tensor_scalar_mul(y[:], y[:], 5.0)
            nc.sync.dma_start(out=out_d[:, :], in_=y[:])
            psC_cm.__exit__(None, None, None)

    nc.compile()
    return nc


# ---------------------------------------------------------------------------
# entry point
# ---------------------------------------------------------------------------

_CACHE = {}
LAST_RUN_SECONDS = None


def kernel(**inputs):
    import time
    from concourse.bass_utils import run_bass_kernel_spmd

    global LAST_RUN_SECONDS
    edge_index = np.asarray(inputs["edge_index"])
    edge_type = np.asarray(inputs["edge_type"])

    import hashlib

    key = hashlib.md5(edge_index.tobytes() + edge_type.tobytes()).hexdigest()
    if key not in _CACHE:
        cores, common = preprocess(edge_index, edge_type)
        nc = build_graph(common)
        _CACHE[key] = (cores, common, nc)
    cores, common, nc = _CACHE[key]
    in_maps = build_core_inputs(inputs, cores, common)

    t0 = time.time()
    res = run_bass_kernel_spmd(nc, in_maps, core_ids=list(range(NCORES)))
    LAST_RUN_SECONDS = time.time() - t0

    out = np.empty((N, 1), np.float32)
    for c in range(NCORES):
        o = res.results[c]["out"]  # [128, NCH]
        ranks = cores[c]["rank"]
        out[c * NS : (c + 1) * NS, 0] = o[ranks % 128, ranks // 128]
    return out


if __name__ == "__main__":
    import reference

    inputs = reference.setup_inputs()
    expected = np.asarray(reference.reference(**inputs))
    got = kernel(**{k: np.asarray(v) for k, v in inputs.items()})
    rel = np.linalg.norm(got - expected) / np.linalg.norm(expected)
    print(f"Relative error: {rel:.3e}")



# revision 11
# speedup vs baseline: 1.0395x; 1.0395x over previous
"""Trainium2 Bass kernel for the RGCN message-passing model (nn_Actor_12094627905962).

Strategy (8 NeuronCores, dst-sharded):
  - Each core owns a contiguous range of NS=12500 destination nodes and all
    edges pointing into them.
  - Layer 1 is fully gather-free: for every (pass, chunk) slot the host packs
    a relation-masked input column U[s] = concat_r[mask_r*x_aug(src) ;
    mask_r*ea_aug(e)] (56 rows).  On device md = relu(Wblk^T U) (768-dim via 6
    matmuls) and msg = Wstack^T md accumulates straight into the PSUM tile of
    the destination chunk (relu(0)=0 makes dummy slots free).  This replaces
    the n-table, its AllGather, the e-term table and all L1 indirect gathers.
  - Layer 2 (max aggregation): per-relation transformed tables of the layer-1
    output are AllGathered, then multi-pass [128,1] indirect gathers fill
    per-(dst,rel) slot accumulators with DVE max folds (as before, minus the
    per-chunk index copies).
  - stage2 (sum of per-relation maxes) and the dense head are unchanged.

All float math runs on device; the host side only shards/permutes inputs and
builds the masked U matrix + u32 gather index tables.
"""

import sys

if "/opt/trn_rl_repo" not in sys.path:
    sys.path.insert(0, "/opt/trn_rl_repo")

import numpy as np
import ml_dtypes

BF = ml_dtypes.bfloat16

N = 100_000
E = 1_600_000
R = 8
NCORES = 8
NS = N // NCORES  # 12500
D = 64
NSP = 12800  # node positions padded (100 chunks of 128)
NCH = NSP // 128  # 100
SEG = R * NSP + 2  # rows per rank segment in gathered H table (+zero,+neg rows)
ZROW = R * NSP
NROW = R * NSP + 1
NEGBIG = -1.0e30
L2_SLAB_CHUNKS = 234


def _ceil(a, b):
    return -(-a // b)


# ---------------------------------------------------------------------------
# host-side preprocessing
# ---------------------------------------------------------------------------


def preprocess(edge_index, edge_type):
    """Pure index preprocessing. Returns per-core structures + common sizes."""
    src = np.asarray(edge_index[0], np.int64)
    dst = np.asarray(edge_index[1], np.int64)
    rel = np.asarray(edge_type, np.int64)

    core_of_edge = dst // NS
    cores = []
    for c in range(NCORES):
        m = np.nonzero(core_of_edge == c)[0]
        cores.append({"eids": m, "s": src[m], "d": dst[m] - c * NS, "r": rel[m]})

    # global rank: per core, nodes ordered by layer-1 in-degree (desc)
    grank = np.empty(N, np.int64)
    for c in range(NCORES):
        deg = np.bincount(cores[c]["d"], minlength=NS)
        order = np.argsort(-deg, kind="stable")
        rank = np.empty(NS, np.int64)
        rank[order] = np.arange(NS)
        cores[c]["deg"] = deg
        cores[c]["rank"] = rank
        grank[c * NS : (c + 1) * NS] = rank

    for c in range(NCORES):
        cc = cores[c]
        s, d, r = cc["s"], cc["d"], cc["r"]
        ne = len(s)
        rank = cc["rank"]

        # --- L1: per-node edge slot (j-th edge of its dst) ---
        dorder = np.argsort(d, kind="stable")
        ds = d[dorder]
        starts = np.searchsorted(ds, np.arange(NS))
        j1 = np.arange(ne) - starts[ds]
        cc["l1_edge"] = dorder
        cc["l1_j"] = j1
        cc["l1_pos"] = rank[ds]
        cc["maxd1"] = int(cc["deg"].max()) if ne else 0
        degsorted = -np.sort(-cc["deg"])
        cc["cnt1"] = np.array(
            [int((degsorted >= j + 1).sum()) for j in range(cc["maxd1"])], np.int64
        )

        # --- L2: (dst, rel) groups ---
        g = d * R + r
        gorder = np.argsort(g, kind="stable")
        gs = g[gorder]
        uniq, uidx, ucnt = np.unique(gs, return_index=True, return_counts=True)
        ngroups = len(uniq)
        grank2 = np.argsort(-ucnt, kind="stable")
        slot_of_u = np.empty(ngroups, np.int64)
        slot_of_u[grank2] = np.arange(ngroups)
        gid_of_edge = np.searchsorted(uniq, gs)
        j2 = np.arange(ne) - uidx[gid_of_edge]
        cc["l2_edge"] = gorder
        cc["l2_j"] = j2
        cc["l2_slot"] = slot_of_u[gid_of_edge]
        cc["l2_cnt"] = ucnt[gid_of_edge]  # group size per sorted edge
        cc["l2_ngroups"] = ngroups
        cc["l2_n2"] = int((ucnt >= 2).sum())  # slotted (non-singleton) groups
        cc["l2_ucnt"] = ucnt
        cc["l2_uidx"] = uidx
        cc["maxd2"] = int(ucnt.max()) if ne else 0
        csorted = -np.sort(-ucnt)
        cc["cnt2"] = np.array(
            [int((csorted >= j + 1).sum()) for j in range(cc["maxd2"])], np.int64
        )
        cc["s2_dst"] = uniq // R
        cc["s2_slot"] = slot_of_u

    # ---- common (max-over-cores) sizes ----
    maxd1 = max(c["maxd1"] for c in cores)
    maxd2 = max(c["maxd2"] for c in cores)
    cmax1 = np.zeros(maxd1, np.int64)
    cmax2 = np.zeros(maxd2, np.int64)
    for c in cores:
        cmax1[: c["maxd1"]] = np.maximum(cmax1[: c["maxd1"]], c["cnt1"])
        cmax2[: c["maxd2"]] = np.maximum(cmax2[: c["maxd2"]], c["cnt2"])
    C1 = np.array([_ceil(int(x), 128) for x in cmax1], np.int64)
    SLOTMAX = max(c["l2_n2"] for c in cores)  # only count>=2 groups get slots
    CH2 = _ceil(SLOTMAX, 128)
    C2 = np.array([_ceil(int(x), 128) for x in cmax2], np.int64)
    C2[0] = CH2  # pass 0 (bypass) covers all slotted groups incl. dummy slots

    # stage2 chunk extents: furthest rank position with > k nonempty rels
    CS2 = np.zeros(8, np.int64)
    for c in range(NCORES):
        cc = cores[c]
        s2d = cc["s2_dst"]
        rank = cc["rank"]
        dorder2 = np.argsort(s2d, kind="stable")
        sd = s2d[dorder2]
        st = np.searchsorted(sd, np.arange(NS))
        en = np.searchsorted(sd, np.arange(NS), side="right")
        nrels = en - st
        cc["s2_dorder"] = dorder2
        cc["s2_st"] = st
        cc["s2_nrels"] = nrels
        for k in range(8):
            m = np.nonzero(nrels > k)[0]
            if len(m):
                CS2[k] = max(CS2[k], _ceil(int(rank[m].max()) + 1, 128))
    ncol_l1 = int(C1.sum())
    ncol_l2 = int(C2.sum())
    col_l2 = np.concatenate([[0], np.cumsum(C2)])[:-1]
    col_s2 = ncol_l2 + np.concatenate([[0], np.cumsum(CS2)])[:-1]
    TC = ncol_l2 + int(CS2.sum())
    ucol = np.concatenate([[0], np.cumsum(C1)])[:-1]  # U column-chunk base per pass

    common = {
        "maxd1": maxd1,
        "maxd2": maxd2,
        "C1": C1,
        "C2": C2,
        "CS2": CS2,
        "CH2": CH2,
        "SLOTMAX": SLOTMAX,
        "grank": grank,
        "TC": TC,
        "col_l2": col_l2,
        "col_s2": col_s2,
        "ucol": ucol,
        "ncol_l1": ncol_l1,
        "SL1": ncol_l1 * 128,
    }

    # ---- per-core index arenas (L2 passes + stage2 only) ----
    for c in range(NCORES):
        cc = cores[c]
        s, d, r = cc["s"], cc["d"], cc["r"]
        arena = np.zeros((128, TC), np.uint32)

        def _htab_row(rr, ss):
            return (ss // NS) * SEG + rr * NSP + grank[ss]

        ndummy = np.uint32(NROW)  # core0 segment -big row

        def _fill(colbase, nchunks, positions, rows, dummy):
            block = np.full(nchunks * 128, dummy, np.uint32)
            block[positions] = rows.astype(np.uint32)
            arena[:, colbase : colbase + nchunks] = block.reshape(nchunks, 128).T

        # L2 passes (singleton groups are skipped; stage2 reads them directly)
        ge, gj, gslot, gcnt = cc["l2_edge"], cc["l2_j"], cc["l2_slot"], cc["l2_cnt"]
        for j in range(maxd2):
            m = (gj == j) & (gcnt >= 2)
            nchunk = int(C2[j])
            pos = gslot[m]
            eids = ge[m]
            hrows = _htab_row(r[eids], s[eids])
            _fill(int(col_l2[j]), nchunk, pos, hrows, ndummy)

        # stage2 passes: node at position p gets its k-th group's value row --
        # slot row (count>=2, in the s2buf region at QOFF) or the single
        # edge's H-table row (singleton groups)
        QOFF = NCORES * SEG
        s2slot = cc["s2_slot"]
        ucnt2 = cc["l2_ucnt"]
        uidx2 = cc["l2_uidx"]
        ge2 = cc["l2_edge"]
        rank = cc["rank"]
        dorder2 = cc["s2_dorder"]
        st = cc["s2_st"]
        nrels = cc["s2_nrels"]
        for k in range(8):
            m = nrels > k
            pos = rank[np.nonzero(m)[0]]
            uids = dorder2[st[m] + k]
            first_e = ge2[uidx2[uids]]  # first edge of each group
            single_rows = _htab_row(r[first_e], s[first_e])
            rows = np.where(
                ucnt2[uids] >= 2, QOFF + s2slot[uids], single_rows
            ).astype(np.uint32)
            _fill(int(col_s2[k]), int(CS2[k]), pos, rows, np.uint32(QOFF + CH2 * 128))

        cc["arena"] = arena

        # --- U column index per edge (L1 masked input) ---
        # sorted-edge t: edge id l1_edge[t], slot = (ucol[j]+0)*128 + l1_pos[t]
        cc["uslot"] = ucol[cc["l1_j"]] * 128 + cc["l1_pos"]

    return cores, common


def build_core_inputs(inputs, cores, common):
    """Per-core numpy input dict."""
    x = np.asarray(inputs["x"], np.float32)
    ea = np.asarray(inputs["edge_attr"], np.float32)
    om = np.asarray(inputs["omega"], np.float32)

    f = lambda k: np.asarray(inputs[k], np.float32)
    Wn, bn = f("Wn"), f("bn")
    We, be = f("We"), f("be")
    Wo, bo = f("Wo"), f("bo")
    W1, Wroot1, b1 = f("W1"), f("Wroot1"), f("b1")
    W2, Wroot2, b2 = f("W2"), f("Wroot2"), f("b2")
    Wagg, bagg = f("Wagg"), f("bagg")
    Wc, bc = f("Wc"), f("bc")

    # f32 weight pack [10, 64]: Wnx 0:4, Wox 4:7, Wex (unused rows kept for layout)
    wf = np.zeros((10, 64), np.float32)
    wf[0:3, :] = Wn
    wf[3, :] = bn
    wf[4:6, :] = Wo
    wf[6, :] = bo

    # bf16 pack rows (same offsets as before where still used):
    #   0:512   Wstack n-part (64r+i -> W1[r, i, :])
    #   512:768 Wstack e-part (512+32r+i -> W1[r, 64+i, :])
    #   768:1280 W2
    #   1280:1345 Wroot1 + b1
    #   1345:1410 Wroot2 + b2
    #   1410:1475 WaggA + bagg
    #   1475:1539 WaggB
    #   1539:1604 Wc + bc (col 0)
    wb = np.zeros((1604, 64), np.float32)
    wb[0:512] = W1[:, :64, :].reshape(512, 64)
    wb[512:768] = W1[:, 64:96, :].reshape(256, 64)
    wb[768:1280] = W2.reshape(512, 64)
    wb[1280:1344] = Wroot1
    wb[1344] = b1
    wb[1345:1409] = Wroot2
    wb[1409] = b2
    wb[1410:1474] = Wagg[:64]
    wb[1474] = bagg
    wb[1475:1539] = Wagg[64:]
    wb[1539:1603, 0] = Wc[:, 0]
    wb[1603, 0] = bc[0]
    wb = wb.astype(BF)

    # block-diagonal masked-encoder weights [56, 768]
    Wn_aug = np.vstack([Wn, bn[None, :]])  # [4, 64]
    We_aug = np.vstack([We, be[None, :]])  # [3, 32]
    wq = np.zeros((56, 768), np.float32)
    for r in range(R):
        wq[4 * r : 4 * r + 4, 64 * r : 64 * r + 64] = Wn_aug
        wq[32 + 3 * r : 32 + 3 * r + 3, 512 + 32 * r : 512 + 32 * r + 32] = We_aug
    wq = wq.astype(BF)

    SL1 = common["SL1"]
    in_maps = []
    for c in range(NCORES):
        cc = cores[c]
        rank = cc["rank"]
        inv = np.argsort(rank)  # position -> node
        xT = np.zeros((4, NSP), np.float32)
        xT[:3, :NS] = x[c * NS : (c + 1) * NS][inv].T
        xT[3, :] = 1.0
        omT = np.zeros((3, NSP), np.float32)
        omT[:2, :NS] = om[c * NS : (c + 1) * NS][inv].T
        omT[2, :] = 1.0

        # masked U matrix [56, SL1]
        U = np.zeros((56, SL1), np.float32)
        eids = cc["eids"][cc["l1_edge"]]  # global edge id per sorted edge
        ssrc = cc["s"][cc["l1_edge"]]
        rr = cc["r"][cc["l1_edge"]]
        sl = cc["uslot"]
        U[4 * rr + 0, sl] = x[ssrc, 0]
        U[4 * rr + 1, sl] = x[ssrc, 1]
        U[4 * rr + 2, sl] = x[ssrc, 2]
        U[4 * rr + 3, sl] = 1.0
        U[32 + 3 * rr + 0, sl] = ea[eids, 0]
        U[32 + 3 * rr + 1, sl] = ea[eids, 1]
        U[32 + 3 * rr + 2, sl] = 1.0

        in_maps.append(
            {
                "xT": xT,
                "omT": omT,
                "wf": wf,
                "wb": wb,
                "wq": wq,
                "U": U.astype(BF),
                "idxs": cc["arena"],
            }
        )
    return in_maps


# ---------------------------------------------------------------------------
# device graph
# ---------------------------------------------------------------------------


SKIP_L1MD = False
SKIP_L2 = False
SKIP_S2 = False
SIM_NO_COLL = False  # replace AllGather with local DMA (TimelineSim only)


def build_graph(common):
    import concourse.bacc as bacc
    import concourse.bass as bass
    import concourse.mybir as mybir
    from concourse.tile import TileContext
    from concourse.masks import make_identity

    fp32 = mybir.dt.float32
    bf16 = mybir.dt.bfloat16
    u32 = mybir.dt.uint32
    AX = mybir.AluOpType

    C1, C2 = common["C1"], common["C2"]
    CS2 = common["CS2"]
    maxd1, maxd2 = common["maxd1"], common["maxd2"]
    col_l2, col_s2 = common["col_l2"], common["col_s2"]
    ucol = common["ucol"]
    CH2 = common["CH2"]
    TC = common["TC"]
    SL1 = common["SL1"]
    S2ROWS = CH2 * 128 + 1

    nc = bacc.Bacc(None, target_bir_lowering=False, num_swdge_queues=4)

    xT = nc.dram_tensor("xT", [4, NSP], fp32, kind="ExternalInput")
    omT = nc.dram_tensor("omT", [3, NSP], fp32, kind="ExternalInput")
    wf = nc.dram_tensor("wf", [10, 64], fp32, kind="ExternalInput")
    wb = nc.dram_tensor("wb", [1604, 64], bf16, kind="ExternalInput")
    wq_d = nc.dram_tensor("wq", [56, 768], bf16, kind="ExternalInput")
    U_d = nc.dram_tensor("U", [56, SL1], bf16, kind="ExternalInput")
    idxs_d = nc.dram_tensor("idxs", [128, TC], u32, kind="ExternalInput")
    out_d = nc.dram_tensor("out", [128, NCH], fp32, kind="ExternalOutput")

    hT_in = nc.dram_tensor("hT_in", [64, NSP], bf16)
    hT_all = nc.dram_tensor("hT_all", [NCORES * 64, NSP], bf16, addr_space="Shared")
    QOFF = NCORES * SEG  # s2buf region lives after the H-table in one tensor
    agH_out = nc.dram_tensor("agH_out", [QOFF + S2ROWS, 64], bf16)

    groups = [list(range(NCORES))]

    # passes per chunk for L1
    npass_of_chunk = [sum(1 for j in range(maxd1) if C1[j] > ch) for ch in range(NCH)]

    with TileContext(nc) as tc:
        with (
            tc.tile_pool(name="persist", bufs=1) as pp,
            tc.tile_pool(name="work", bufs=3) as wk,
        ):
            # ---- persistent tiles ----
            idxs = pp.tile([128, TC], u32)
            nc.sync.dma_start(out=idxs[:], in_=idxs_d[:, :])

            wnx = pp.tile([4, 64], fp32)
            nc.sync.dma_start(out=wnx[:], in_=wf[0:4, :])
            wox = pp.tile([3, 64], fp32)
            nc.sync.dma_start(out=wox[:], in_=wf[4:7, :])
            wqt = pp.tile([56, 768], bf16)
            nc.sync.dma_start(out=wqt[:], in_=wq_d[:, :])
            wstk = pp.tile([128, 6, 64], bf16)
            for k in range(6):
                nc.sync.dma_start(
                    out=wstk[:, k, :], in_=wb[128 * k : 128 * (k + 1), :]
                )
            w2a = pp.tile([64, R * 64], bf16)
            for rr in range(R):
                nc.sync.dma_start(
                    out=w2a[:, rr * 64 : (rr + 1) * 64],
                    in_=wb[768 + rr * 64 : 768 + (rr + 1) * 64, :],
                )
            wroot1 = pp.tile([65, 64], bf16)
            nc.sync.dma_start(out=wroot1[:], in_=wb[1280:1345, :])
            wroot2 = pp.tile([65, 64], bf16)
            nc.sync.dma_start(out=wroot2[:], in_=wb[1345:1410, :])
            wagga = pp.tile([65, 64], bf16)
            nc.sync.dma_start(out=wagga[:], in_=wb[1410:1475, :])
            waggb = pp.tile([64, 64], bf16)
            nc.sync.dma_start(out=waggb[:], in_=wb[1475:1539, :])
            wcx = pp.tile([65, 64], bf16)
            nc.sync.dma_start(out=wcx[:], in_=wb[1539:1604, :])

            ident = pp.tile([128, 128], bf16)
            make_identity(nc, ident[:])

            zrow = pp.tile([1, 64], bf16)
            nc.vector.memset(zrow[:], 0.0)
            nrow = pp.tile([1, 64], bf16)
            nc.vector.memset(nrow[:], NEGBIG)

            # ---- own-shard encoder nT (feature-major) ----
            sc_enc = nc.enter_named_scope("enc", False)
            nT = pp.tile([65, NSP], bf16)
            nc.vector.memset(nT[64:65, :], 1.0)
            with tc.tile_pool(name="psA", bufs=2, space="PSUM") as psA:
                for b in range(NSP // 512):
                    sl = slice(b * 512, (b + 1) * 512)
                    xch = wk.tile([4, 512], fp32, tag="xch")
                    nc.sync.dma_start(out=xch[:], in_=xT[:, sl])
                    p1 = psA.tile([64, 512], fp32, space="PSUM", tag="pa")
                    nc.tensor.matmul(p1[:], lhsT=wnx[:], rhs=xch[:], start=True, stop=True)
                    nc.scalar.activation(
                        nT[0:64, sl], p1[:], mybir.ActivationFunctionType.Relu
                    )

            nc.leave_named_scope("enc", sc_enc[0], False)
            # ---- layer 1: masked dense compute (j-outer, SBUF accumulator) ----
            sc_l1 = nc.enter_named_scope("L1", False)
            hT = pp.tile([65, NSP], bf16)
            nc.vector.memset(hT[64:65, :], 1.0)
            UBLK = 32  # chunks of U per staged load
            with tc.tile_pool(name="psMd", bufs=2, space="PSUM") as psMd, tc.tile_pool(
                name="psAcc", bufs=2, space="PSUM"
            ) as psAcc, tc.tile_pool(
                name="psT", bufs=2, space="PSUM"
            ) as psT, tc.tile_pool(name="uw", bufs=3) as uw, tc.tile_pool(
                name="uv", bufs=8
            ) as uv, tc.tile_pool(name="a1p", bufs=1) as a1p:
                acc1 = a1p.tile([128, NCH, 64], fp32)
                nc.vector.memset(acc1[:], 0.0)
                for jj in range(0 if SKIP_L1MD else maxd1):
                    nchj = int(C1[jj])
                    for b0 in range(0, nchj, UBLK):
                        b1 = min(nchj, b0 + UBLK)
                        ucb = (int(ucol[jj]) + b0) * 128
                        ust = uw.tile([56, UBLK * 128], bf16, tag="ust")
                        nc.sync.dma_start(
                            out=ust[:, 0 : (b1 - b0) * 128],
                            in_=U_d[:, ucb : ucb + (b1 - b0) * 128],
                        )
                        for ch in range(b0, b1):
                            us = slice((ch - b0) * 128, (ch - b0 + 1) * 128)
                            mdp1 = psMd.tile([128, 384], fp32, space="PSUM", tag="md1")
                            mdp2 = psMd.tile([128, 384], fp32, space="PSUM", tag="md2")
                            for k in range(6):
                                tgt = mdp1 if k < 3 else mdp2
                                fs = slice((k % 3) * 128, (k % 3) * 128 + 128)
                                nc.tensor.matmul(
                                    tgt[:, fs],
                                    lhsT=wqt[:, 128 * k : 128 * (k + 1)],
                                    rhs=ust[:, us],
                                    start=True,
                                    stop=True,
                                )
                            mds = uv.tile([128, 6, 128], bf16, tag="mds")
                            nc.scalar.activation(
                                mds[:, 0:3, :].rearrange("p a f -> p (a f)"),
                                mdp1[:],
                                mybir.ActivationFunctionType.Relu,
                            )
                            nc.scalar.activation(
                                mds[:, 3:6, :].rearrange("p a f -> p (a f)"),
                                mdp2[:],
                                mybir.ActivationFunctionType.Relu,
                            )
                            msg = psAcc.tile([128, 64], fp32, space="PSUM", tag="msg")
                            for k in range(6):
                                nc.tensor.matmul(
                                    msg[:],
                                    lhsT=mds[:, k, :],
                                    rhs=wstk[:, k, :],
                                    start=(k == 0),
                                    stop=(k == 5),
                                )
                            nc.vector.tensor_tensor(
                                acc1[:, ch, :], acc1[:, ch, :], msg[:], op=AX.add
                            )
                # finalize: root term + accumulated messages, relu, transpose
                for ch in range(NCH):
                    sl = slice(ch * 128, (ch + 1) * 128)
                    p = psAcc.tile([128, 64], fp32, space="PSUM", tag="msg")
                    nc.tensor.matmul(
                        p[:], lhsT=nT[:, sl], rhs=wroot1[:], start=True, stop=True
                    )
                    nc.vector.tensor_tensor(p[:], p[:], acc1[:, ch, :], op=AX.add)
                    hch = wk.tile([128, 64], bf16, tag="hch")
                    nc.vector.tensor_scalar_max(hch[:], p[:], 0.0)
                    pt = psT.tile([64, 128], bf16, space="PSUM", tag="pt")
                    nc.tensor.transpose(pt[:], hch[:], ident[:])
                    nc.scalar.activation(
                        hT[0:64, sl], pt[:], mybir.ActivationFunctionType.Copy
                    )

            nc.leave_named_scope("L1", sc_l1[0], False)
            # ---- AllGather raw hT, then build the 8-relation table locally ----
            sc_ag = nc.enter_named_scope("AG", False)
            nc.sync.dma_start(out=hT_in[:, :], in_=hT[0:64, :])
            if SIM_NO_COLL:
                for c in range(NCORES):
                    nc.sync.dma_start(
                        out=hT_all[c * 64 : (c + 1) * 64, :], in_=hT_in[:, :]
                    )
            else:
                nc.gpsimd.collective_compute(
                    "AllGather",
                    mybir.AluOpType.bypass,
                    replica_groups=groups,
                    ins=[hT_in[:, :]],
                    outs=[hT_all[:, :]],
                )
            nc.leave_named_scope("AG", sc_ag[0], False)
            sc_ht = nc.enter_named_scope("Htab", False)
            with tc.tile_pool(name="psB", bufs=2, space="PSUM") as psB, tc.tile_pool(
                name="hw", bufs=3
            ) as hw:
                for c in range(NCORES):
                    for b in range(NSP // 512):
                        hg = hw.tile([64, 512], bf16, tag="hg")
                        nc.sync.dma_start(
                            out=hg[:],
                            in_=hT_all[c * 64 : (c + 1) * 64, b * 512 : (b + 1) * 512],
                        )
                        for q in range(4):
                            ch = b * 4 + q
                            p = psB.tile([128, R * 64], fp32, space="PSUM", tag="p")
                            nc.tensor.matmul(
                                p[:],
                                lhsT=hg[:, q * 128 : (q + 1) * 128],
                                rhs=w2a[:],
                                start=True,
                                stop=True,
                            )
                            stagH = wk.tile([128, R, 64], bf16, tag="stagH")
                            nc.scalar.activation(
                                stagH[:].rearrange("p r f -> p (r f)"),
                                p[:],
                                mybir.ActivationFunctionType.Copy,
                            )
                            nc.sync.dma_start(
                                out=agH_out[c * SEG : c * SEG + R * NSP, :]
                                .rearrange("(r ch p) f -> ch p r f", p=128, ch=NCH)[ch],
                                in_=stagH[:],
                            )
            nc.sync.dma_start(out=agH_out[ZROW : ZROW + 1, :], in_=zrow[:])
            nc.sync.dma_start(out=agH_out[NROW : NROW + 1, :], in_=nrow[:])

            nc.leave_named_scope("Htab", sc_ht[0], False)
            # ---- layer-2 max: per-chunk staged gathers + DVE max folds (slabs) ----
            sc_l2 = nc.enter_named_scope("L2max", False)
            L2SLAB = L2_SLAB_CHUNKS
            nslabs = _ceil(CH2, L2SLAB)
            with tc.tile_pool(name="l2p", bufs=2) as l2p, tc.tile_pool(
                name="gp2", bufs=32
            ) as gp2:
                for sb in range(nslabs):
                    sch0 = sb * L2SLAB
                    sch1 = min(CH2, sch0 + L2SLAB)
                    a2s = l2p.tile([128, L2SLAB, 64], bf16, tag="a2s")
                    for j in range(0 if SKIP_L2 else maxd2):
                        c0 = int(col_l2[j])
                        hi = min(int(C2[j]), sch1)
                        for ch in range(sch0, hi):
                            if j == 0:
                                # pass 0 covers every slot: gather in place,
                                # no memset / staging / max-fold needed
                                gih = nc.gpsimd.indirect_dma_start(
                                    out=a2s[:, ch - sch0, :],
                                    out_offset=None,
                                    in_=agH_out[:, :],
                                    in_offset=bass.IndirectOffsetOnAxis(
                                        ap=idxs[:, c0 + ch : c0 + ch + 1], axis=0
                                    ),
                                )
                                gih.ins.queue = f"qPoolDynamic{ch % 4 or ''}"
                                continue
                            stg = gp2.tile([128, 64], bf16, tag="stg")
                            gih = nc.gpsimd.indirect_dma_start(
                                out=stg[:, :],
                                out_offset=None,
                                in_=agH_out[:, :],
                                in_offset=bass.IndirectOffsetOnAxis(
                                    ap=idxs[:, c0 + ch : c0 + ch + 1], axis=0
                                ),
                            )
                            gih.ins.queue = f"qPoolDynamic{ch % 4 or ''}" 
                            nc.vector.tensor_tensor(
                                a2s[:, ch - sch0, :],
                                a2s[:, ch - sch0, :],
                                stg[:],
                                op=AX.max,
                            )
                    nc.sync.dma_start(
                        out=agH_out[QOFF + sch0 * 128 : QOFF + sch1 * 128, :].rearrange(
                            "(ch p) f -> p ch f", p=128
                        ),
                        in_=a2s[:, 0 : sch1 - sch0, :],
                    )
            nc.sync.dma_start(
                out=agH_out[QOFF + CH2 * 128 : QOFF + CH2 * 128 + 1, :], in_=zrow[:]
            )

            nc.leave_named_scope("L2max", sc_l2[0], False)
            # ---- stage2: per-node sum of its nonempty-rel maxes ----
            sc_s2 = nc.enter_named_scope("S2", False)
            acc2e = pp.tile([128, NCH, 64], bf16)
            nc.vector.memset(acc2e[:], 0.0)
            with tc.tile_pool(name="gp3", bufs=32) as gp3:
                for ch in range(0 if SKIP_S2 else int(CS2.max())):
                    for k in range(8):
                        if ch >= int(CS2[k]):
                            continue
                        c0 = int(col_s2[k])
                        if k == 0:
                            # k=0 covers every node (zero row for edgeless
                            # ones): gather in place over the memset
                            gih = nc.gpsimd.indirect_dma_start(
                                out=acc2e[:, ch, :],
                                out_offset=None,
                                in_=agH_out[:, :],
                                in_offset=bass.IndirectOffsetOnAxis(
                                    ap=idxs[:, c0 + ch : c0 + ch + 1], axis=0
                                ),
                            )
                            gih.ins.queue = f"qPoolDynamic{(ch + k) % 4 or ''}"
                            continue
                        stg = gp3.tile([128, 64], bf16, tag="stg")
                        gih = nc.gpsimd.indirect_dma_start(
                            out=stg[:, :],
                            out_offset=None,
                            in_=agH_out[:, :],
                            in_offset=bass.IndirectOffsetOnAxis(
                                ap=idxs[:, c0 + ch : c0 + ch + 1], axis=0
                            ),
                        )
                        gih.ins.queue = f"qPoolDynamic{(ch + k) % 4 or ''}"
                        nc.vector.tensor_tensor(
                            acc2e[:, ch, :], acc2e[:, ch, :], stg[:], op=AX.add
                        )

            nc.leave_named_scope("S2", sc_s2[0], False)
            # ---- h2 = relu(root2 + acc2e); head ----
            sc_hd = nc.enter_named_scope("head", False)
            y = pp.tile([128, NCH], fp32)
            psC_cm = tc.tile_pool(name="psC", bufs=4, space="PSUM")
            psC = psC_cm.__enter__()
            for ch in range(NCH):
                sl = slice(ch * 128, (ch + 1) * 128)
                p = psC.tile([128, 64], fp32, space="PSUM", tag="pc")
                nc.tensor.matmul(
                    p[:], lhsT=hT[:, sl], rhs=wroot2[:], start=True, stop=True
                )
                nc.vector.tensor_tensor(p[:], p[:], acc2e[:, ch, :], op=AX.add)
                h2 = wk.tile([128, 64], bf16, tag="h2")
                nc.vector.tensor_scalar_max(h2[:], p[:], 0.0)
                pt = psC.tile([64, 128], bf16, space="PSUM", tag="pc")
                nc.tensor.transpose(pt[:], h2[:], ident[:])
                h2T = wk.tile([65, 128], bf16, tag="h2T")
                nc.scalar.activation(
                    h2T[0:64, :], pt[:], mybir.ActivationFunctionType.Copy
                )
                nc.vector.memset(h2T[64:65, :], 1.0)
                if ch % 4 == 0:
                    sl4 = slice(ch * 128, min((ch + 4) * 128, NSP))
                    omch = wk.tile([3, 512], fp32, tag="omch")
                    nc.sync.dma_start(out=omch[:, 0 : sl4.stop - sl4.start], in_=omT[:, sl4])
                    po4 = psC.tile([64, 512], fp32, space="PSUM", tag="po4")
                    nc.tensor.matmul(
                        po4[:], lhsT=wox[:], rhs=omch[:], start=True, stop=True
                    )
                    oT4 = wk.tile([64, 512], bf16, tag="oT4")
                    nc.scalar.activation(
                        oT4[:], po4[:], mybir.ActivationFunctionType.Relu
                    )
                oTc = oT4[:, (ch % 4) * 128 : (ch % 4 + 1) * 128]
                p3 = psC.tile([128, 64], fp32, space="PSUM", tag="pc")
                nc.tensor.matmul(p3[:], lhsT=h2T[:], rhs=wagga[:], start=True, stop=False)
                nc.tensor.matmul(
                    p3[:], lhsT=oTc, rhs=waggb[:], start=False, stop=True
                )
                h3 = wk.tile([128, 64], bf16, tag="h3")
                nc.vector.tensor_scalar_max(h3[:], p3[:], 0.0)
                pt2 = psC.tile([64, 128], bf16, space="PSUM", tag="pc")
                nc.tensor.transpose(pt2[:], h3[:], ident[:])
                h3T = wk.tile([65, 128], bf16, tag="h3T")
                nc.scalar.activation(
                    h3T[0:64, :], pt2[:], mybir.ActivationFunctionType.Copy
                )
                nc.vector.memset(h3T[64:65, :], 1.0)
                py = psC.tile([128, 64], fp32, space="PSUM", tag="pc")
                nc.tensor.matmul(py[:], lhsT=h3T[:], rhs=wcx[:], start=True, stop=True)
                nc.scalar.activation(
                    y[:, ch : ch + 1],
                    py[:, 0:1],
                    mybir.ActivationFunctionType.Tanh,
                )
            nc.vector.tensor_scalar_mul(y[:], y[:], 5.0)
            nc.sync.dma_start(out=out_d[:, :], in_=y[:])
            psC_cm.__exit__(None, None, None)
            nc.leave_named_scope("head", sc_hd[0], False)

    nc.compile()
    return nc


# ---------------------------------------------------------------------------
# entry point
# ---------------------------------------------------------------------------

_CACHE = {}
LAST_RUN_SECONDS = None


def kernel(**inputs):
    import time
    from concourse.bass_utils import run_bass_kernel_spmd

    global LAST_RUN_SECONDS
    edge_index = np.asarray(inputs["edge_index"])
    edge_type = np.asarray(inputs["edge_type"])

    import hashlib

    key = hashlib.md5(edge_index.tobytes() + edge_type.tobytes()).hexdigest()
    if key not in _CACHE:
        cores, common = preprocess(edge_index, edge_type)
        nc = build_graph(common)
        _CACHE[key] = (cores, common, nc)
    cores, common, nc = _CACHE[key]
    in_maps = build_core_inputs(inputs, cores, common)

    t0 = time.time()
    res = run_bass_kernel_spmd(nc, in_maps, core_ids=list(range(NCORES)))
    LAST_RUN_SECONDS = time.time() - t0

    out = np.empty((N, 1), np.float32)
    for c in range(NCORES):
        o = res.results[c]["out"]  # [128, NCH]
        ranks = cores[c]["rank"]
        out[c * NS : (c + 1) * NS, 0] = o[ranks % 128, ranks // 128]
    return out


if __name__ == "__main__":
    import reference

    inputs = reference.setup_inputs()
    expected = np.asarray(reference.reference(**inputs))
    got = kernel(**{k: np.asarray(v) for k, v in inputs.items()})
    rel = np.linalg.norm(got - expected) / np.linalg.norm(expected)
    print(f"Relative error: {rel:.3e}")



# revision 12
# speedup vs baseline: 1.0670x; 1.0264x over previous
"""Trainium2 Bass kernel for the RGCN message-passing model (nn_Actor_12094627905962).

Strategy (8 NeuronCores, dst-sharded):
  - Each core owns a contiguous range of NS=12500 destination nodes and all
    edges pointing into them.
  - Layer 1 is fully gather-free: for every (pass, chunk) slot the host packs
    a relation-masked input column U[s] = concat_r[mask_r*x_aug(src) ;
    mask_r*ea_aug(e)] (56 rows).  On device md = relu(Wblk^T U) (768-dim via 6
    matmuls) and msg = Wstack^T md accumulates straight into the PSUM tile of
    the destination chunk (relu(0)=0 makes dummy slots free).  This replaces
    the n-table, its AllGather, the e-term table and all L1 indirect gathers.
  - Layer 2 (max aggregation): per-relation transformed tables of the layer-1
    output are AllGathered, then multi-pass [128,1] indirect gathers fill
    per-(dst,rel) slot accumulators with DVE max folds (as before, minus the
    per-chunk index copies).
  - stage2 (sum of per-relation maxes) and the dense head are unchanged.

All float math runs on device; the host side only shards/permutes inputs and
builds the masked U matrix + u32 gather index tables.
"""

import sys

if "/opt/trn_rl_repo" not in sys.path:
    sys.path.insert(0, "/opt/trn_rl_repo")

import numpy as np
import ml_dtypes

BF = ml_dtypes.bfloat16

N = 100_000
E = 1_600_000
R = 8
NCORES = 8
NS = N // NCORES  # 12500
D = 64
NSP = 12800  # node positions padded (100 chunks of 128)
NCH = NSP // 128  # 100
SEG = R * NSP + 2  # rows per rank segment in gathered H table (+zero,+neg rows)
ZROW = R * NSP
NROW = R * NSP + 1
NEGBIG = -1.0e30
L2_SLAB_CHUNKS = 234


def _ceil(a, b):
    return -(-a // b)


# ---------------------------------------------------------------------------
# host-side preprocessing
# ---------------------------------------------------------------------------


def preprocess(edge_index, edge_type):
    """Pure index preprocessing. Returns per-core structures + common sizes."""
    src = np.asarray(edge_index[0], np.int64)
    dst = np.asarray(edge_index[1], np.int64)
    rel = np.asarray(edge_type, np.int64)

    core_of_edge = dst // NS
    cores = []
    for c in range(NCORES):
        m = np.nonzero(core_of_edge == c)[0]
        cores.append({"eids": m, "s": src[m], "d": dst[m] - c * NS, "r": rel[m]})

    # global rank: per core, nodes ordered by layer-1 in-degree (desc)
    grank = np.empty(N, np.int64)
    for c in range(NCORES):
        deg = np.bincount(cores[c]["d"], minlength=NS)
        order = np.argsort(-deg, kind="stable")
        rank = np.empty(NS, np.int64)
        rank[order] = np.arange(NS)
        cores[c]["deg"] = deg
        cores[c]["rank"] = rank
        grank[c * NS : (c + 1) * NS] = rank

    for c in range(NCORES):
        cc = cores[c]
        s, d, r = cc["s"], cc["d"], cc["r"]
        ne = len(s)
        rank = cc["rank"]

        # --- L1: per-node edge slot (j-th edge of its dst) ---
        dorder = np.argsort(d, kind="stable")
        ds = d[dorder]
        starts = np.searchsorted(ds, np.arange(NS))
        j1 = np.arange(ne) - starts[ds]
        cc["l1_edge"] = dorder
        cc["l1_j"] = j1
        cc["l1_pos"] = rank[ds]
        cc["maxd1"] = int(cc["deg"].max()) if ne else 0
        degsorted = -np.sort(-cc["deg"])
        cc["cnt1"] = np.array(
            [int((degsorted >= j + 1).sum()) for j in range(cc["maxd1"])], np.int64
        )

        # --- L2: (dst, rel) groups ---
        g = d * R + r
        gorder = np.argsort(g, kind="stable")
        gs = g[gorder]
        uniq, uidx, ucnt = np.unique(gs, return_index=True, return_counts=True)
        ngroups = len(uniq)
        grank2 = np.argsort(-ucnt, kind="stable")
        slot_of_u = np.empty(ngroups, np.int64)
        slot_of_u[grank2] = np.arange(ngroups)
        gid_of_edge = np.searchsorted(uniq, gs)
        j2 = np.arange(ne) - uidx[gid_of_edge]
        cc["l2_edge"] = gorder
        cc["l2_j"] = j2
        cc["l2_slot"] = slot_of_u[gid_of_edge]
        cc["l2_cnt"] = ucnt[gid_of_edge]  # group size per sorted edge
        cc["l2_ngroups"] = ngroups
        cc["l2_n2"] = int((ucnt >= 2).sum())  # slotted (non-singleton) groups
        cc["l2_ucnt"] = ucnt
        cc["l2_uidx"] = uidx
        cc["maxd2"] = int(ucnt.max()) if ne else 0
        csorted = -np.sort(-ucnt)
        cc["cnt2"] = np.array(
            [int((csorted >= j + 1).sum()) for j in range(cc["maxd2"])], np.int64
        )
        cc["s2_dst"] = uniq // R
        cc["s2_slot"] = slot_of_u

    # ---- common (max-over-cores) sizes ----
    maxd1 = max(c["maxd1"] for c in cores)
    maxd2 = max(c["maxd2"] for c in cores)
    cmax1 = np.zeros(maxd1, np.int64)
    cmax2 = np.zeros(maxd2, np.int64)
    for c in cores:
        cmax1[: c["maxd1"]] = np.maximum(cmax1[: c["maxd1"]], c["cnt1"])
        cmax2[: c["maxd2"]] = np.maximum(cmax2[: c["maxd2"]], c["cnt2"])
    C1 = np.array([_ceil(int(x), 128) for x in cmax1], np.int64)
    SLOTMAX = max(c["l2_n2"] for c in cores)  # only count>=2 groups get slots
    CH2 = _ceil(SLOTMAX, 128)
    C2 = np.array([_ceil(int(x), 128) for x in cmax2], np.int64)
    C2[0] = CH2  # pass 0 (bypass) covers all slotted groups incl. dummy slots

    # stage2 chunk extents: furthest rank position with > k nonempty rels
    CS2 = np.zeros(8, np.int64)
    for c in range(NCORES):
        cc = cores[c]
        s2d = cc["s2_dst"]
        rank = cc["rank"]
        dorder2 = np.argsort(s2d, kind="stable")
        sd = s2d[dorder2]
        st = np.searchsorted(sd, np.arange(NS))
        en = np.searchsorted(sd, np.arange(NS), side="right")
        nrels = en - st
        cc["s2_dorder"] = dorder2
        cc["s2_st"] = st
        cc["s2_nrels"] = nrels
        for k in range(8):
            m = np.nonzero(nrels > k)[0]
            if len(m):
                CS2[k] = max(CS2[k], _ceil(int(rank[m].max()) + 1, 128))
    ncol_l1 = int(C1.sum())
    ncol_l2 = int(C2.sum())
    col_l2 = np.concatenate([[0], np.cumsum(C2)])[:-1]
    col_s2 = ncol_l2 + np.concatenate([[0], np.cumsum(CS2)])[:-1]
    TC = ncol_l2 + int(CS2.sum())
    ucol = np.concatenate([[0], np.cumsum(C1)])[:-1]  # U column-chunk base per pass

    common = {
        "maxd1": maxd1,
        "maxd2": maxd2,
        "C1": C1,
        "C2": C2,
        "CS2": CS2,
        "CH2": CH2,
        "SLOTMAX": SLOTMAX,
        "grank": grank,
        "TC": TC,
        "col_l2": col_l2,
        "col_s2": col_s2,
        "ucol": ucol,
        "ncol_l1": ncol_l1,
        "SL1": ncol_l1 * 128,
    }

    # ---- per-core index arenas (L2 passes + stage2 only) ----
    for c in range(NCORES):
        cc = cores[c]
        s, d, r = cc["s"], cc["d"], cc["r"]
        arena = np.zeros((128, TC), np.uint32)

        def _htab_row(rr, ss):
            return (ss // NS) * SEG + rr * NSP + grank[ss]

        ndummy = np.uint32(NROW)  # core0 segment -big row

        def _fill(colbase, nchunks, positions, rows, dummy):
            block = np.full(nchunks * 128, dummy, np.uint32)
            block[positions] = rows.astype(np.uint32)
            arena[:, colbase : colbase + nchunks] = block.reshape(nchunks, 128).T

        # L2 passes (singleton groups are skipped; stage2 reads them directly)
        ge, gj, gslot, gcnt = cc["l2_edge"], cc["l2_j"], cc["l2_slot"], cc["l2_cnt"]
        for j in range(maxd2):
            m = (gj == j) & (gcnt >= 2)
            nchunk = int(C2[j])
            pos = gslot[m]
            eids = ge[m]
            hrows = _htab_row(r[eids], s[eids])
            _fill(int(col_l2[j]), nchunk, pos, hrows, ndummy)

        # stage2 passes: node at position p gets its k-th group's value row --
        # slot row (count>=2, in the s2buf region at QOFF) or the single
        # edge's H-table row (singleton groups)
        QOFF = NCORES * SEG
        s2slot = cc["s2_slot"]
        ucnt2 = cc["l2_ucnt"]
        uidx2 = cc["l2_uidx"]
        ge2 = cc["l2_edge"]
        rank = cc["rank"]
        dorder2 = cc["s2_dorder"]
        st = cc["s2_st"]
        nrels = cc["s2_nrels"]
        for k in range(8):
            m = nrels > k
            pos = rank[np.nonzero(m)[0]]
            uids = dorder2[st[m] + k]
            first_e = ge2[uidx2[uids]]  # first edge of each group
            single_rows = _htab_row(r[first_e], s[first_e])
            rows = np.where(
                ucnt2[uids] >= 2, QOFF + s2slot[uids], single_rows
            ).astype(np.uint32)
            _fill(int(col_s2[k]), int(CS2[k]), pos, rows, np.uint32(QOFF + CH2 * 128))

        cc["arena"] = arena

        # --- U column index per edge (L1 masked input) ---
        # sorted-edge t: edge id l1_edge[t], slot = (ucol[j]+0)*128 + l1_pos[t]
        cc["uslot"] = ucol[cc["l1_j"]] * 128 + cc["l1_pos"]

    return cores, common


def build_core_inputs(inputs, cores, common):
    """Per-core numpy input dict."""
    x = np.asarray(inputs["x"], np.float32)
    ea = np.asarray(inputs["edge_attr"], np.float32)
    om = np.asarray(inputs["omega"], np.float32)

    f = lambda k: np.asarray(inputs[k], np.float32)
    Wn, bn = f("Wn"), f("bn")
    We, be = f("We"), f("be")
    Wo, bo = f("Wo"), f("bo")
    W1, Wroot1, b1 = f("W1"), f("Wroot1"), f("b1")
    W2, Wroot2, b2 = f("W2"), f("Wroot2"), f("b2")
    Wagg, bagg = f("Wagg"), f("bagg")
    Wc, bc = f("Wc"), f("bc")

    # f32 weight pack [10, 64]: Wnx 0:4, Wox 4:7, Wex (unused rows kept for layout)
    wf = np.zeros((10, 64), np.float32)
    wf[0:3, :] = Wn
    wf[3, :] = bn
    wf[4:6, :] = Wo
    wf[6, :] = bo

    # bf16 pack rows (same offsets as before where still used):
    #   0:512   Wstack n-part (64r+i -> W1[r, i, :])
    #   512:768 Wstack e-part (512+32r+i -> W1[r, 64+i, :])
    #   768:1280 W2
    #   1280:1345 Wroot1 + b1
    #   1345:1410 Wroot2 + b2
    #   1410:1475 WaggA + bagg
    #   1475:1539 WaggB
    #   1539:1604 Wc + bc (col 0)
    wb = np.zeros((1604, 64), np.float32)
    wb[0:512] = W1[:, :64, :].reshape(512, 64)
    wb[512:768] = W1[:, 64:96, :].reshape(256, 64)
    wb[768:1280] = W2.reshape(512, 64)
    wb[1280:1344] = Wroot1
    wb[1344] = b1
    wb[1345:1409] = Wroot2
    wb[1409] = b2
    wb[1410:1474] = Wagg[:64]
    wb[1474] = bagg
    wb[1475:1539] = Wagg[64:]
    wb[1539:1603, 0] = Wc[:, 0]
    wb[1603, 0] = bc[0]
    wb = wb.astype(BF)

    # block-diagonal masked-encoder weights [56, 768]
    Wn_aug = np.vstack([Wn, bn[None, :]])  # [4, 64]
    We_aug = np.vstack([We, be[None, :]])  # [3, 32]
    wq = np.zeros((56, 768), np.float32)
    for r in range(R):
        wq[4 * r : 4 * r + 4, 64 * r : 64 * r + 64] = Wn_aug
        wq[32 + 3 * r : 32 + 3 * r + 3, 512 + 32 * r : 512 + 32 * r + 32] = We_aug
    wq = wq.astype(BF)

    SL1 = common["SL1"]
    in_maps = []
    for c in range(NCORES):
        cc = cores[c]
        rank = cc["rank"]
        inv = np.argsort(rank)  # position -> node
        xT = np.zeros((4, NSP), np.float32)
        xT[:3, :NS] = x[c * NS : (c + 1) * NS][inv].T
        xT[3, :] = 1.0
        omT = np.zeros((3, NSP), np.float32)
        omT[:2, :NS] = om[c * NS : (c + 1) * NS][inv].T
        omT[2, :] = 1.0

        # masked U matrix [56, SL1]
        U = np.zeros((56, SL1), np.float32)
        eids = cc["eids"][cc["l1_edge"]]  # global edge id per sorted edge
        ssrc = cc["s"][cc["l1_edge"]]
        rr = cc["r"][cc["l1_edge"]]
        sl = cc["uslot"]
        U[4 * rr + 0, sl] = x[ssrc, 0]
        U[4 * rr + 1, sl] = x[ssrc, 1]
        U[4 * rr + 2, sl] = x[ssrc, 2]
        U[4 * rr + 3, sl] = 1.0
        U[32 + 3 * rr + 0, sl] = ea[eids, 0]
        U[32 + 3 * rr + 1, sl] = ea[eids, 1]
        U[32 + 3 * rr + 2, sl] = 1.0

        in_maps.append(
            {
                "xT": xT,
                "omT": omT,
                "wf": wf,
                "wb": wb,
                "wq": wq,
                "U": U.astype(BF),
                "idxs": cc["arena"],
            }
        )
    return in_maps


# ---------------------------------------------------------------------------
# device graph
# ---------------------------------------------------------------------------


SKIP_L1MD = False
SKIP_L2 = False
SKIP_S2 = False
SIM_NO_COLL = False  # replace AllGather with local DMA (TimelineSim only)


def build_graph(common):
    import concourse.bacc as bacc
    import concourse.bass as bass
    import concourse.mybir as mybir
    from concourse.tile import TileContext
    from concourse.masks import make_identity

    fp32 = mybir.dt.float32
    bf16 = mybir.dt.bfloat16
    u32 = mybir.dt.uint32
    AX = mybir.AluOpType

    C1, C2 = common["C1"], common["C2"]
    CS2 = common["CS2"]
    maxd1, maxd2 = common["maxd1"], common["maxd2"]
    col_l2, col_s2 = common["col_l2"], common["col_s2"]
    ucol = common["ucol"]
    CH2 = common["CH2"]
    TC = common["TC"]
    SL1 = common["SL1"]
    S2ROWS = CH2 * 128 + 1

    nc = bacc.Bacc(None, target_bir_lowering=False, num_swdge_queues=4)

    xT = nc.dram_tensor("xT", [4, NSP], fp32, kind="ExternalInput")
    omT = nc.dram_tensor("omT", [3, NSP], fp32, kind="ExternalInput")
    wf = nc.dram_tensor("wf", [10, 64], fp32, kind="ExternalInput")
    wb = nc.dram_tensor("wb", [1604, 64], bf16, kind="ExternalInput")
    wq_d = nc.dram_tensor("wq", [56, 768], bf16, kind="ExternalInput")
    U_d = nc.dram_tensor("U", [56, SL1], bf16, kind="ExternalInput")
    idxs_d = nc.dram_tensor("idxs", [128, TC], u32, kind="ExternalInput")
    out_d = nc.dram_tensor("out", [128, NCH], fp32, kind="ExternalOutput")

    hT_in = nc.dram_tensor("hT_in", [64, NSP], bf16)
    hT_all = nc.dram_tensor("hT_all", [NCORES * 64, NSP], bf16, addr_space="Shared")
    QOFF = NCORES * SEG  # s2buf region lives after the H-table in one tensor
    agH_out = nc.dram_tensor("agH_out", [QOFF + S2ROWS, 64], bf16)

    groups = [list(range(NCORES))]

    # passes per chunk for L1
    npass_of_chunk = [sum(1 for j in range(maxd1) if C1[j] > ch) for ch in range(NCH)]

    with TileContext(nc) as tc:
        with (
            tc.tile_pool(name="persist", bufs=1) as pp,
            tc.tile_pool(name="work", bufs=3) as wk,
        ):
            # ---- persistent tiles ----
            idxs = pp.tile([128, TC], u32)
            nc.sync.dma_start(out=idxs[:], in_=idxs_d[:, :])

            wnx = pp.tile([4, 64], fp32)
            nc.sync.dma_start(out=wnx[:], in_=wf[0:4, :])
            wox = pp.tile([3, 64], fp32)
            nc.sync.dma_start(out=wox[:], in_=wf[4:7, :])
            wqt = pp.tile([56, 768], bf16)
            nc.sync.dma_start(out=wqt[:], in_=wq_d[:, :])
            wstk = pp.tile([128, 6, 64], bf16)
            for k in range(6):
                nc.sync.dma_start(
                    out=wstk[:, k, :], in_=wb[128 * k : 128 * (k + 1), :]
                )
            w2a = pp.tile([64, R * 64], bf16)
            for rr in range(R):
                nc.sync.dma_start(
                    out=w2a[:, rr * 64 : (rr + 1) * 64],
                    in_=wb[768 + rr * 64 : 768 + (rr + 1) * 64, :],
                )
            wroot1 = pp.tile([65, 64], bf16)
            nc.sync.dma_start(out=wroot1[:], in_=wb[1280:1345, :])
            wroot2 = pp.tile([65, 64], bf16)
            nc.sync.dma_start(out=wroot2[:], in_=wb[1345:1410, :])
            wagga = pp.tile([65, 64], bf16)
            nc.sync.dma_start(out=wagga[:], in_=wb[1410:1475, :])
            waggb = pp.tile([64, 64], bf16)
            nc.sync.dma_start(out=waggb[:], in_=wb[1475:1539, :])
            wcx = pp.tile([65, 64], bf16)
            nc.sync.dma_start(out=wcx[:], in_=wb[1539:1604, :])

            ident = pp.tile([128, 128], bf16)
            make_identity(nc, ident[:])

            zrow = pp.tile([1, 64], bf16)
            nc.vector.memset(zrow[:], 0.0)
            nrow = pp.tile([1, 64], bf16)
            nc.vector.memset(nrow[:], NEGBIG)

            # ---- own-shard encoder nT (feature-major) ----
            sc_enc = nc.enter_named_scope("enc", False)
            nT = pp.tile([65, NSP], bf16)
            nc.vector.memset(nT[64:65, :], 1.0)
            with tc.tile_pool(name="psA", bufs=2, space="PSUM") as psA:
                for b in range(NSP // 512):
                    sl = slice(b * 512, (b + 1) * 512)
                    xch = wk.tile([4, 512], fp32, tag="xch")
                    nc.sync.dma_start(out=xch[:], in_=xT[:, sl])
                    p1 = psA.tile([64, 512], fp32, space="PSUM", tag="pa")
                    nc.tensor.matmul(p1[:], lhsT=wnx[:], rhs=xch[:], start=True, stop=True)
                    nc.scalar.activation(
                        nT[0:64, sl], p1[:], mybir.ActivationFunctionType.Relu
                    )

            nc.leave_named_scope("enc", sc_enc[0], False)
            # ---- layer 1: masked dense compute (j-outer, SBUF accumulator) ----
            sc_l1 = nc.enter_named_scope("L1", False)
            hT = pp.tile([65, NSP], bf16)
            nc.vector.memset(hT[64:65, :], 1.0)
            UBLK = 32  # chunks of U per staged load
            with tc.tile_pool(name="psMd", bufs=2, space="PSUM") as psMd, tc.tile_pool(
                name="psAcc", bufs=2, space="PSUM"
            ) as psAcc, tc.tile_pool(
                name="psT", bufs=2, space="PSUM"
            ) as psT, tc.tile_pool(name="uw", bufs=3) as uw, tc.tile_pool(
                name="uv", bufs=8
            ) as uv, tc.tile_pool(name="a1p", bufs=1) as a1p:
                acc1 = a1p.tile([128, NCH, 64], fp32)
                nc.vector.memset(acc1[:], 0.0)
                for jj in range(0 if SKIP_L1MD else maxd1):
                    nchj = int(C1[jj])
                    for b0 in range(0, nchj, UBLK):
                        b1 = min(nchj, b0 + UBLK)
                        ucb = (int(ucol[jj]) + b0) * 128
                        ust = uw.tile([56, UBLK * 128], bf16, tag="ust")
                        nc.sync.dma_start(
                            out=ust[:, 0 : (b1 - b0) * 128],
                            in_=U_d[:, ucb : ucb + (b1 - b0) * 128],
                        )
                        for ch in range(b0, b1):
                            us = slice((ch - b0) * 128, (ch - b0 + 1) * 128)
                            mdp1 = psMd.tile([128, 384], fp32, space="PSUM", tag="md1")
                            mdp2 = psMd.tile([128, 384], fp32, space="PSUM", tag="md2")
                            for k in range(6):
                                tgt = mdp1 if k < 3 else mdp2
                                fs = slice((k % 3) * 128, (k % 3) * 128 + 128)
                                nc.tensor.matmul(
                                    tgt[:, fs],
                                    lhsT=wqt[:, 128 * k : 128 * (k + 1)],
                                    rhs=ust[:, us],
                                    start=True,
                                    stop=True,
                                )
                            mds = uv.tile([128, 6, 128], bf16, tag="mds")
                            nc.scalar.activation(
                                mds[:, 0:3, :].rearrange("p a f -> p (a f)"),
                                mdp1[:],
                                mybir.ActivationFunctionType.Relu,
                            )
                            nc.scalar.activation(
                                mds[:, 3:6, :].rearrange("p a f -> p (a f)"),
                                mdp2[:],
                                mybir.ActivationFunctionType.Relu,
                            )
                            msg = psAcc.tile([128, 64], fp32, space="PSUM", tag="msg")
                            for k in range(6):
                                nc.tensor.matmul(
                                    msg[:],
                                    lhsT=mds[:, k, :],
                                    rhs=wstk[:, k, :],
                                    start=(k == 0),
                                    stop=(k == 5),
                                )
                            nc.vector.tensor_tensor(
                                acc1[:, ch, :], acc1[:, ch, :], msg[:], op=AX.add
                            )
                # finalize: root term + accumulated messages, relu, transpose
                for ch in range(NCH):
                    sl = slice(ch * 128, (ch + 1) * 128)
                    p = psAcc.tile([128, 64], fp32, space="PSUM", tag="msg")
                    nc.tensor.matmul(
                        p[:], lhsT=nT[:, sl], rhs=wroot1[:], start=True, stop=True
                    )
                    nc.vector.tensor_tensor(p[:], p[:], acc1[:, ch, :], op=AX.add)
                    hch = wk.tile([128, 64], bf16, tag="hch")
                    nc.vector.tensor_scalar_max(hch[:], p[:], 0.0)
                    pt = psT.tile([64, 128], bf16, space="PSUM", tag="pt")
                    nc.tensor.transpose(pt[:], hch[:], ident[:])
                    nc.scalar.activation(
                        hT[0:64, sl], pt[:], mybir.ActivationFunctionType.Copy
                    )

            nc.leave_named_scope("L1", sc_l1[0], False)
            # ---- AllGather raw hT, then build the 8-relation table locally ----
            sc_ag = nc.enter_named_scope("AG", False)
            nc.sync.dma_start(out=hT_in[:, :], in_=hT[0:64, :])
            if SIM_NO_COLL:
                for c in range(NCORES):
                    nc.sync.dma_start(
                        out=hT_all[c * 64 : (c + 1) * 64, :], in_=hT_in[:, :]
                    )
            else:
                nc.gpsimd.collective_compute(
                    "AllGather",
                    mybir.AluOpType.bypass,
                    replica_groups=groups,
                    ins=[hT_in[:, :]],
                    outs=[hT_all[:, :]],
                )
            nc.leave_named_scope("AG", sc_ag[0], False)
            sc_ht = nc.enter_named_scope("Htab", False)
            with tc.tile_pool(name="psB", bufs=2, space="PSUM") as psB, tc.tile_pool(
                name="hw", bufs=3
            ) as hw:
                for c in range(NCORES):
                    for b in range(NSP // 512):
                        hg = hw.tile([64, 512], bf16, tag="hg")
                        nc.sync.dma_start(
                            out=hg[:],
                            in_=hT_all[c * 64 : (c + 1) * 64, b * 512 : (b + 1) * 512],
                        )
                        for q in range(4):
                            ch = b * 4 + q
                            p = psB.tile([128, R * 64], fp32, space="PSUM", tag="p")
                            nc.tensor.matmul(
                                p[:],
                                lhsT=hg[:, q * 128 : (q + 1) * 128],
                                rhs=w2a[:],
                                start=True,
                                stop=True,
                            )
                            stagH = wk.tile([128, R, 64], bf16, tag="stagH")
                            nc.scalar.activation(
                                stagH[:].rearrange("p r f -> p (r f)"),
                                p[:],
                                mybir.ActivationFunctionType.Copy,
                            )
                            nc.sync.dma_start(
                                out=agH_out[c * SEG : c * SEG + R * NSP, :]
                                .rearrange("(r ch p) f -> ch p r f", p=128, ch=NCH)[ch],
                                in_=stagH[:],
                            )
            nc.sync.dma_start(out=agH_out[ZROW : ZROW + 1, :], in_=zrow[:])
            nc.sync.dma_start(out=agH_out[NROW : NROW + 1, :], in_=nrow[:])

            nc.leave_named_scope("Htab", sc_ht[0], False)
            # ---- layer-2 max: per-chunk staged gathers + DVE max folds (slabs) ----
            sc_l2 = nc.enter_named_scope("L2max", False)
            L2SLAB = L2_SLAB_CHUNKS
            nslabs = _ceil(CH2, L2SLAB)
            with tc.tile_pool(name="l2p", bufs=2) as l2p, tc.tile_pool(
                name="gp2", bufs=32
            ) as gp2:
                for sb in range(nslabs):
                    sch0 = sb * L2SLAB
                    sch1 = min(CH2, sch0 + L2SLAB)
                    a2s = l2p.tile([128, L2SLAB, 64], bf16, tag="a2s")
                    if SKIP_L2:
                        nc.vector.memset(a2s[:], 0.0)
                    for j in range(0 if SKIP_L2 else maxd2):
                        c0 = int(col_l2[j])
                        hi = min(int(C2[j]), sch1)
                        for ch in range(sch0, hi):
                            if j == 0:
                                # pass 0 covers every slot: gather in place,
                                # no memset / staging / max-fold needed
                                gih = nc.gpsimd.indirect_dma_start(
                                    out=a2s[:, ch - sch0, :],
                                    out_offset=None,
                                    in_=agH_out[:, :],
                                    in_offset=bass.IndirectOffsetOnAxis(
                                        ap=idxs[:, c0 + ch : c0 + ch + 1], axis=0
                                    ),
                                )
                                gih.ins.queue = f"qPoolDynamic{ch % 4 or ''}"
                                continue
                            stg = gp2.tile([128, 64], bf16, tag="stg")
                            gih = nc.gpsimd.indirect_dma_start(
                                out=stg[:, :],
                                out_offset=None,
                                in_=agH_out[:, :],
                                in_offset=bass.IndirectOffsetOnAxis(
                                    ap=idxs[:, c0 + ch : c0 + ch + 1], axis=0
                                ),
                            )
                            gih.ins.queue = f"qPoolDynamic{ch % 4 or ''}" 
                            nc.vector.tensor_tensor(
                                a2s[:, ch - sch0, :],
                                a2s[:, ch - sch0, :],
                                stg[:],
                                op=AX.max,
                            )
                    nc.sync.dma_start(
                        out=agH_out[QOFF + sch0 * 128 : QOFF + sch1 * 128, :].rearrange(
                            "(ch p) f -> p ch f", p=128
                        ),
                        in_=a2s[:, 0 : sch1 - sch0, :],
                    )
            nc.sync.dma_start(
                out=agH_out[QOFF + CH2 * 128 : QOFF + CH2 * 128 + 1, :], in_=zrow[:]
            )

            nc.leave_named_scope("L2max", sc_l2[0], False)
            # ---- stage2: per-node sum of its nonempty-rel maxes ----
            sc_s2 = nc.enter_named_scope("S2", False)
            acc2e = pp.tile([128, NCH, 64], bf16)
            nc.vector.memset(acc2e[:], 0.0)
            with tc.tile_pool(name="gp3", bufs=32) as gp3:
                for ch in range(0 if SKIP_S2 else int(CS2.max())):
                    for k in range(8):
                        if ch >= int(CS2[k]):
                            continue
                        c0 = int(col_s2[k])
                        if k == 0:
                            # k=0 covers every node (zero row for edgeless
                            # ones): gather in place over the memset
                            gih = nc.gpsimd.indirect_dma_start(
                                out=acc2e[:, ch, :],
                                out_offset=None,
                                in_=agH_out[:, :],
                                in_offset=bass.IndirectOffsetOnAxis(
                                    ap=idxs[:, c0 + ch : c0 + ch + 1], axis=0
                                ),
                            )
                            gih.ins.queue = f"qPoolDynamic{(ch + k) % 4 or ''}"
                            continue
                        stg = gp3.tile([128, 64], bf16, tag="stg")
                        gih = nc.gpsimd.indirect_dma_start(
                            out=stg[:, :],
                            out_offset=None,
                            in_=agH_out[:, :],
                            in_offset=bass.IndirectOffsetOnAxis(
                                ap=idxs[:, c0 + ch : c0 + ch + 1], axis=0
                            ),
                        )
                        gih.ins.queue = f"qPoolDynamic{(ch + k) % 4 or ''}"
                        nc.vector.tensor_tensor(
                            acc2e[:, ch, :], acc2e[:, ch, :], stg[:], op=AX.add
                        )

            nc.leave_named_scope("S2", sc_s2[0], False)
            # ---- h2 = relu(root2 + acc2e); head ----
            sc_hd = nc.enter_named_scope("head", False)
            y = pp.tile([128, NCH], fp32)
            psC_cm = tc.tile_pool(name="psC", bufs=4, space="PSUM")
            psC = psC_cm.__enter__()
            for ch in range(NCH):
                sl = slice(ch * 128, (ch + 1) * 128)
                p = psC.tile([128, 64], fp32, space="PSUM", tag="pc")
                nc.tensor.matmul(
                    p[:], lhsT=hT[:, sl], rhs=wroot2[:], start=True, stop=True
                )
                nc.vector.tensor_tensor(p[:], p[:], acc2e[:, ch, :], op=AX.add)
                h2 = wk.tile([128, 64], bf16, tag="h2")
                nc.vector.tensor_scalar_max(h2[:], p[:], 0.0)
                pt = psC.tile([64, 128], bf16, space="PSUM", tag="pc")
                nc.tensor.transpose(pt[:], h2[:], ident[:])
                h2T = wk.tile([65, 128], bf16, tag="h2T")
                nc.scalar.activation(
                    h2T[0:64, :], pt[:], mybir.ActivationFunctionType.Copy
                )
                nc.vector.memset(h2T[64:65, :], 1.0)
                if ch % 4 == 0:
                    sl4 = slice(ch * 128, min((ch + 4) * 128, NSP))
                    omch = wk.tile([3, 512], fp32, tag="omch")
                    nc.sync.dma_start(out=omch[:, 0 : sl4.stop - sl4.start], in_=omT[:, sl4])
                    po4 = psC.tile([64, 512], fp32, space="PSUM", tag="po4")
                    nc.tensor.matmul(
                        po4[:], lhsT=wox[:], rhs=omch[:], start=True, stop=True
                    )
                    oT4 = wk.tile([64, 512], bf16, tag="oT4")
                    nc.scalar.activation(
                        oT4[:], po4[:], mybir.ActivationFunctionType.Relu
                    )
                oTc = oT4[:, (ch % 4) * 128 : (ch % 4 + 1) * 128]
                p3 = psC.tile([128, 64], fp32, space="PSUM", tag="pc")
                nc.tensor.matmul(p3[:], lhsT=h2T[:], rhs=wagga[:], start=True, stop=False)
                nc.tensor.matmul(
                    p3[:], lhsT=oTc, rhs=waggb[:], start=False, stop=True
                )
                h3 = wk.tile([128, 64], bf16, tag="h3")
                nc.vector.tensor_scalar_max(h3[:], p3[:], 0.0)
                pt2 = psC.tile([64, 128], bf16, space="PSUM", tag="pc")
                nc.tensor.transpose(pt2[:], h3[:], ident[:])
                h3T = wk.tile([65, 128], bf16, tag="h3T")
                nc.scalar.activation(
                    h3T[0:64, :], pt2[:], mybir.ActivationFunctionType.Copy
                )
                nc.vector.memset(h3T[64:65, :], 1.0)
                py = psC.tile([128, 64], fp32, space="PSUM", tag="pc")
                nc.tensor.matmul(py[:], lhsT=h3T[:], rhs=wcx[:], start=True, stop=True)
                nc.scalar.activation(
                    y[:, ch : ch + 1],
                    py[:, 0:1],
                    mybir.ActivationFunctionType.Tanh,
                )
            nc.vector.tensor_scalar_mul(y[:], y[:], 5.0)
            nc.sync.dma_start(out=out_d[:, :], in_=y[:])
            psC_cm.__exit__(None, None, None)
            nc.leave_named_scope("head", sc_hd[0], False)

    nc.compile()
    return nc


# ---------------------------------------------------------------------------
# entry point
# ---------------------------------------------------------------------------

_CACHE = {}
LAST_RUN_SECONDS = None


def kernel(**inputs):
    import time
    from concourse.bass_utils import run_bass_kernel_spmd

    global LAST_RUN_SECONDS
    edge_index = np.asarray(inputs["edge_index"])
    edge_type = np.asarray(inputs["edge_type"])

    import hashlib

    key = hashlib.md5(edge_index.tobytes() + edge_type.tobytes()).hexdigest()
    if key not in _CACHE:
        cores, common = preprocess(edge_index, edge_type)
        nc = build_graph(common)
        _CACHE[key] = (cores, common, nc)
    cores, common, nc = _CACHE[key]
    in_maps = build_core_inputs(inputs, cores, common)

    t0 = time.time()
    res = run_bass_kernel_spmd(nc, in_maps, core_ids=list(range(NCORES)))
    LAST_RUN_SECONDS = time.time() - t0

    out = np.empty((N, 1), np.float32)
    for c in range(NCORES):
        o = res.results[c]["out"]  # [128, NCH]
        ranks = cores[c]["rank"]
        out[c * NS : (c + 1) * NS, 0] = o[ranks % 128, ranks // 128]
    return out


if __name__ == "__main__":
    import reference

    inputs = reference.setup_inputs()
    expected = np.asarray(reference.reference(**inputs))
    got = kernel(**{k: np.asarray(v) for k, v in inputs.items()})
    rel = np.linalg.norm(got - expected) / np.linalg.norm(expected)
    print(f"Relative error: {rel:.3e}")



# revision 22
# speedup vs baseline: 1.4035x; 1.3154x over previous
"""Trainium2 Bass kernel for the RGCN message-passing model (nn_Actor_12094627905962).

Strategy (8 NeuronCores, dst-sharded):
  - Each core owns a contiguous range of NS=12500 destination nodes and all
    edges pointing into them.
  - Layer 1 is fully gather-free: for every (pass, chunk) slot the host packs
    a relation-masked input column U[s] = concat_r[mask_r*x_aug(src) ;
    mask_r*ea_aug(e)] (56 rows).  On device md = relu(Wblk^T U) (768-dim via 6
    matmuls) and msg = Wstack^T md accumulates straight into the PSUM tile of
    the destination chunk (relu(0)=0 makes dummy slots free).  This replaces
    the n-table, its AllGather, the e-term table and all L1 indirect gathers.
  - Layer 2 (max aggregation): per-relation transformed tables of the layer-1
    output are AllGathered, then multi-pass [128,1] indirect gathers fill
    per-(dst,rel) slot accumulators with DVE max folds (as before, minus the
    per-chunk index copies).
  - stage2 (sum of per-relation maxes) and the dense head are unchanged.

All float math runs on device; the host side only shards/permutes inputs and
builds the masked U matrix + u32 gather index tables.
"""

import sys

if "/opt/trn_rl_repo" not in sys.path:
    sys.path.insert(0, "/opt/trn_rl_repo")

import numpy as np
import ml_dtypes

BF = ml_dtypes.bfloat16

N = 100_000
E = 1_600_000
R = 8
NCORES = 8
NS = N // NCORES  # 12500
D = 64
NSP = 12800  # node positions padded (100 chunks of 128)
NCH = NSP // 128  # 100
HROWS = NCORES * R * NSP  # H-table rows: row = (c*NSP + pos)*R + r
ZROW = HROWS
NROW = HROWS + 1
QOFF = HROWS + 2  # s2buf region base
NEGBIG = -1.0e30
KB = 48  # gather columns (128-row chunks) per indirect DMA instruction


def _ceil(a, b):
    return -(-a // b)


# ---------------------------------------------------------------------------
# host-side preprocessing
# ---------------------------------------------------------------------------


def preprocess(edge_index, edge_type):
    """Pure index preprocessing. Returns per-core structures + common sizes."""
    src = np.asarray(edge_index[0], np.int64)
    dst = np.asarray(edge_index[1], np.int64)
    rel = np.asarray(edge_type, np.int64)

    core_of_edge = dst // NS
    cores = []
    for c in range(NCORES):
        m = np.nonzero(core_of_edge == c)[0]
        cores.append({"eids": m, "s": src[m], "d": dst[m] - c * NS, "r": rel[m]})

    # global rank: per core, nodes ordered by layer-1 in-degree (desc)
    grank = np.empty(N, np.int64)
    for c in range(NCORES):
        deg = np.bincount(cores[c]["d"], minlength=NS)
        order = np.argsort(-deg, kind="stable")
        rank = np.empty(NS, np.int64)
        rank[order] = np.arange(NS)
        cores[c]["deg"] = deg
        cores[c]["rank"] = rank
        grank[c * NS : (c + 1) * NS] = rank

    for c in range(NCORES):
        cc = cores[c]
        s, d, r = cc["s"], cc["d"], cc["r"]
        ne = len(s)
        rank = cc["rank"]

        # --- L1: per-node edge slot (j-th edge of its dst) ---
        dorder = np.argsort(d, kind="stable")
        ds = d[dorder]
        starts = np.searchsorted(ds, np.arange(NS))
        j1 = np.arange(ne) - starts[ds]
        cc["l1_edge"] = dorder
        cc["l1_j"] = j1
        cc["l1_pos"] = rank[ds]
        cc["maxd1"] = int(cc["deg"].max()) if ne else 0
        degsorted = -np.sort(-cc["deg"])
        cc["cnt1"] = np.array(
            [int((degsorted >= j + 1).sum()) for j in range(cc["maxd1"])], np.int64
        )

        # --- L2: (dst, rel) groups ---
        g = d * R + r
        gorder = np.argsort(g, kind="stable")
        gs = g[gorder]
        uniq, uidx, ucnt = np.unique(gs, return_index=True, return_counts=True)
        ngroups = len(uniq)
        grank2 = np.argsort(-ucnt, kind="stable")
        slot_of_u = np.empty(ngroups, np.int64)
        slot_of_u[grank2] = np.arange(ngroups)
        gid_of_edge = np.searchsorted(uniq, gs)
        j2 = np.arange(ne) - uidx[gid_of_edge]
        cc["l2_edge"] = gorder
        cc["l2_j"] = j2
        cc["l2_slot"] = slot_of_u[gid_of_edge]
        cc["l2_cnt"] = ucnt[gid_of_edge]  # group size per sorted edge
        cc["l2_ngroups"] = ngroups
        cc["l2_n2"] = int((ucnt >= 2).sum())  # slotted (non-singleton) groups
        cc["l2_ucnt"] = ucnt
        cc["l2_uidx"] = uidx
        cc["maxd2"] = int(ucnt.max()) if ne else 0
        csorted = -np.sort(-ucnt)
        cc["cnt2"] = np.array(
            [int((csorted >= j + 1).sum()) for j in range(cc["maxd2"])], np.int64
        )
        cc["s2_dst"] = uniq // R
        cc["s2_slot"] = slot_of_u

    # ---- common (max-over-cores) sizes ----
    maxd1 = max(c["maxd1"] for c in cores)
    maxd2 = max(c["maxd2"] for c in cores)
    cmax1 = np.zeros(maxd1, np.int64)
    cmax2 = np.zeros(maxd2, np.int64)
    for c in cores:
        cmax1[: c["maxd1"]] = np.maximum(cmax1[: c["maxd1"]], c["cnt1"])
        cmax2[: c["maxd2"]] = np.maximum(cmax2[: c["maxd2"]], c["cnt2"])
    C1 = np.array([_ceil(int(x), 128) for x in cmax1], np.int64)
    SLOTMAX = max(c["l2_n2"] for c in cores)  # only count>=2 groups get slots
    CH2 = _ceil(SLOTMAX, 128)
    C2 = np.array([_ceil(int(x), 128) for x in cmax2], np.int64)
    C2[0] = CH2  # pass 0 (bypass) covers all slotted groups incl. dummy slots

    # stage2 chunk extents: furthest rank position with > k nonempty rels
    CS2 = np.zeros(8, np.int64)
    for c in range(NCORES):
        cc = cores[c]
        s2d = cc["s2_dst"]
        rank = cc["rank"]
        dorder2 = np.argsort(s2d, kind="stable")
        sd = s2d[dorder2]
        st = np.searchsorted(sd, np.arange(NS))
        en = np.searchsorted(sd, np.arange(NS), side="right")
        nrels = en - st
        cc["s2_dorder"] = dorder2
        cc["s2_st"] = st
        cc["s2_nrels"] = nrels
        for k in range(8):
            m = np.nonzero(nrels > k)[0]
            if len(m):
                CS2[k] = max(CS2[k], _ceil(int(rank[m].max()) + 1, 128))
    ncol_l1 = int(C1.sum())
    ncol_l2 = int(C2.sum())
    col_l2 = np.concatenate([[0], np.cumsum(C2)])[:-1]
    col_s2 = ncol_l2 + np.concatenate([[0], np.cumsum(CS2)])[:-1]
    TC = ncol_l2 + int(CS2.sum())
    ucol = np.concatenate([[0], np.cumsum(C1)])[:-1]  # U column-chunk base per pass

    common = {
        "maxd1": maxd1,
        "maxd2": maxd2,
        "C1": C1,
        "C2": C2,
        "CS2": CS2,
        "CH2": CH2,
        "SLOTMAX": SLOTMAX,
        "grank": grank,
        "TC": TC,
        "col_l2": col_l2,
        "col_s2": col_s2,
        "ucol": ucol,
        "ncol_l1": ncol_l1,
        "SL1": ncol_l1 * 128,
    }

    # ---- per-core index arenas (L2 passes + stage2 only) ----
    for c in range(NCORES):
        cc = cores[c]
        s, d, r = cc["s"], cc["d"], cc["r"]
        arena = np.zeros((128, TC), np.uint32)

        def _htab_row(rr, ss):
            return ((ss // NS) * NSP + grank[ss]) * R + rr

        ndummy = np.uint32(NROW)  # global -big row

        def _fill(colbase, nchunks, positions, rows, dummy):
            block = np.full(nchunks * 128, dummy, np.uint32)
            block[positions] = rows.astype(np.uint32)
            arena[:, colbase : colbase + nchunks] = block.reshape(nchunks, 128).T

        # L2 passes (singleton groups are skipped; stage2 reads them directly)
        ge, gj, gslot, gcnt = cc["l2_edge"], cc["l2_j"], cc["l2_slot"], cc["l2_cnt"]
        for j in range(maxd2):
            m = (gj == j) & (gcnt >= 2)
            nchunk = int(C2[j])
            pos = gslot[m]
            eids = ge[m]
            hrows = _htab_row(r[eids], s[eids])
            _fill(int(col_l2[j]), nchunk, pos, hrows, ndummy)

        # stage2 passes: node at position p gets its k-th group's value row --
        # slot row (count>=2, in the s2buf region at QOFF) or the single
        # edge's H-table row (singleton groups)
        s2slot = cc["s2_slot"]
        ucnt2 = cc["l2_ucnt"]
        uidx2 = cc["l2_uidx"]
        ge2 = cc["l2_edge"]
        rank = cc["rank"]
        dorder2 = cc["s2_dorder"]
        st = cc["s2_st"]
        nrels = cc["s2_nrels"]
        for k in range(8):
            m = nrels > k
            pos = rank[np.nonzero(m)[0]]
            uids = dorder2[st[m] + k]
            first_e = ge2[uidx2[uids]]  # first edge of each group
            single_rows = _htab_row(r[first_e], s[first_e])
            rows = np.where(
                ucnt2[uids] >= 2, QOFF + s2slot[uids], single_rows
            ).astype(np.uint32)
            _fill(int(col_s2[k]), int(CS2[k]), pos, rows, np.uint32(QOFF + CH2 * 128))

        cc["arena"] = arena

        # --- U column index per edge (L1 masked input) ---
        # sorted-edge t: edge id l1_edge[t], slot = (ucol[j]+0)*128 + l1_pos[t]
        cc["uslot"] = ucol[cc["l1_j"]] * 128 + cc["l1_pos"]

    return cores, common


def build_core_inputs(inputs, cores, common):
    """Per-core numpy input dict."""
    x = np.asarray(inputs["x"], np.float32)
    ea = np.asarray(inputs["edge_attr"], np.float32)
    om = np.asarray(inputs["omega"], np.float32)

    f = lambda k: np.asarray(inputs[k], np.float32)
    Wn, bn = f("Wn"), f("bn")
    We, be = f("We"), f("be")
    Wo, bo = f("Wo"), f("bo")
    W1, Wroot1, b1 = f("W1"), f("Wroot1"), f("b1")
    W2, Wroot2, b2 = f("W2"), f("Wroot2"), f("b2")
    Wagg, bagg = f("Wagg"), f("bagg")
    Wc, bc = f("Wc"), f("bc")

    # f32 weight pack [10, 64]: Wnx 0:4, Wox 4:7, Wex (unused rows kept for layout)
    wf = np.zeros((10, 64), np.float32)
    wf[0:3, :] = Wn
    wf[3, :] = bn
    wf[4:6, :] = Wo
    wf[6, :] = bo

    # bf16 pack rows (same offsets as before where still used):
    #   0:512   Wstack n-part (64r+i -> W1[r, i, :])
    #   512:768 Wstack e-part (512+32r+i -> W1[r, 64+i, :])
    #   768:1280 W2
    #   1280:1345 Wroot1 + b1
    #   1345:1410 Wroot2 + b2
    #   1410:1475 WaggA + bagg
    #   1475:1539 WaggB
    #   1539:1604 Wc + bc (col 0)
    wb = np.zeros((1604, 64), np.float32)
    wb[0:512] = W1[:, :64, :].reshape(512, 64)
    wb[512:768] = W1[:, 64:96, :].reshape(256, 64)
    wb[768:1280] = W2.reshape(512, 64)
    wb[1280:1344] = Wroot1
    wb[1344] = b1
    wb[1345:1409] = Wroot2
    wb[1409] = b2
    wb[1410:1474] = Wagg[:64]
    wb[1474] = bagg
    wb[1475:1539] = Wagg[64:]
    wb[1539:1603, 0] = Wc[:, 0]
    wb[1603, 0] = bc[0]
    wb = wb.astype(BF)

    # block-diagonal masked-encoder weights [56, 768]
    Wn_aug = np.vstack([Wn, bn[None, :]])  # [4, 64]
    We_aug = np.vstack([We, be[None, :]])  # [3, 32]
    wq = np.zeros((56, 768), np.float32)
    for r in range(R):
        wq[4 * r : 4 * r + 4, 64 * r : 64 * r + 64] = Wn_aug
        wq[32 + 3 * r : 32 + 3 * r + 3, 512 + 32 * r : 512 + 32 * r + 32] = We_aug
    wq = wq.astype(BF)

    SL1 = common["SL1"]
    in_maps = []
    for c in range(NCORES):
        cc = cores[c]
        rank = cc["rank"]
        inv = np.argsort(rank)  # position -> node
        xT = np.zeros((4, NSP), np.float32)
        xT[:3, :NS] = x[c * NS : (c + 1) * NS][inv].T
        xT[3, :] = 1.0
        omT = np.zeros((3, NSP), np.float32)
        omT[:2, :NS] = om[c * NS : (c + 1) * NS][inv].T
        omT[2, :] = 1.0

        # masked U matrix [56, SL1]
        U = np.zeros((56, SL1), np.float32)
        eids = cc["eids"][cc["l1_edge"]]  # global edge id per sorted edge
        ssrc = cc["s"][cc["l1_edge"]]
        rr = cc["r"][cc["l1_edge"]]
        sl = cc["uslot"]
        U[4 * rr + 0, sl] = x[ssrc, 0]
        U[4 * rr + 1, sl] = x[ssrc, 1]
        U[4 * rr + 2, sl] = x[ssrc, 2]
        U[4 * rr + 3, sl] = 1.0
        U[32 + 3 * rr + 0, sl] = ea[eids, 0]
        U[32 + 3 * rr + 1, sl] = ea[eids, 1]
        U[32 + 3 * rr + 2, sl] = 1.0

        in_maps.append(
            {
                "xT": xT,
                "omT": omT,
                "wf": wf,
                "wb": wb,
                "wq": wq,
                "U": U.astype(BF),
                "idxs": cc["arena"],
            }
        )
    return in_maps


# ---------------------------------------------------------------------------
# device graph
# ---------------------------------------------------------------------------


SKIP_L1MD = False
SKIP_L2 = False
SKIP_S2 = False
SIM_NO_COLL = False  # replace AllGather with local DMA (TimelineSim only)
USE_CCE_ADD = False  # fold stage2 sums inside the gather DMA (needs HW support)


def build_graph(common):
    import concourse.bacc as bacc
    import concourse.bass as bass
    import concourse.mybir as mybir
    from concourse.tile import TileContext
    from concourse.masks import make_identity

    fp32 = mybir.dt.float32
    bf16 = mybir.dt.bfloat16
    u32 = mybir.dt.uint32
    AX = mybir.AluOpType

    C1, C2 = common["C1"], common["C2"]
    CS2 = common["CS2"]
    maxd1, maxd2 = common["maxd1"], common["maxd2"]
    col_l2, col_s2 = common["col_l2"], common["col_s2"]
    ucol = common["ucol"]
    CH2 = common["CH2"]
    TC = common["TC"]
    SL1 = common["SL1"]
    S2ROWS = CH2 * 128 + 1

    nc = bacc.Bacc(None, target_bir_lowering=False, num_swdge_queues=4)

    xT = nc.dram_tensor("xT", [4, NSP], fp32, kind="ExternalInput")
    omT = nc.dram_tensor("omT", [3, NSP], fp32, kind="ExternalInput")
    wf = nc.dram_tensor("wf", [10, 64], fp32, kind="ExternalInput")
    wb = nc.dram_tensor("wb", [1604, 64], bf16, kind="ExternalInput")
    wq_d = nc.dram_tensor("wq", [56, 768], bf16, kind="ExternalInput")
    U_d = nc.dram_tensor("U", [56, SL1], bf16, kind="ExternalInput")
    idxs_d = nc.dram_tensor("idxs", [128, TC], u32, kind="ExternalInput")
    out_d = nc.dram_tensor("out", [128, NCH], fp32, kind="ExternalOutput")

    hT_in = nc.dram_tensor("hT_in", [64, NSP], bf16)
    hT_all = nc.dram_tensor("hT_all", [NCORES * 64, NSP], bf16, addr_space="Shared")
    # H-table row (c*NSP + pos)*R + r, then zero row, -big row, s2buf region
    agH_out = nc.dram_tensor("agH_out", [QOFF + S2ROWS, 64], bf16)

    groups = [list(range(NCORES))]

    # passes per chunk for L1
    npass_of_chunk = [sum(1 for j in range(maxd1) if C1[j] > ch) for ch in range(NCH)]

    with TileContext(nc) as tc:
        with (
            tc.tile_pool(name="persist", bufs=1) as pp,
            tc.tile_pool(name="work", bufs=3) as wk,
        ):
            # ---- persistent tiles ----
            idxs = pp.tile([128, TC], u32)
            nc.sync.dma_start(out=idxs[:], in_=idxs_d[:, :])

            wnx = pp.tile([4, 64], fp32)
            nc.sync.dma_start(out=wnx[:], in_=wf[0:4, :])
            wox = pp.tile([3, 64], fp32)
            nc.sync.dma_start(out=wox[:], in_=wf[4:7, :])
            wqt = pp.tile([56, 768], bf16)
            nc.sync.dma_start(out=wqt[:], in_=wq_d[:, :])
            wstk = pp.tile([128, 6, 64], bf16)
            for k in range(6):
                nc.sync.dma_start(
                    out=wstk[:, k, :], in_=wb[128 * k : 128 * (k + 1), :]
                )
            w2a = pp.tile([64, R * 64], bf16)
            for rr in range(R):
                nc.sync.dma_start(
                    out=w2a[:, rr * 64 : (rr + 1) * 64],
                    in_=wb[768 + rr * 64 : 768 + (rr + 1) * 64, :],
                )
            wroot1 = pp.tile([65, 64], bf16)
            nc.sync.dma_start(out=wroot1[:], in_=wb[1280:1345, :])
            wroot2 = pp.tile([65, 64], bf16)
            nc.sync.dma_start(out=wroot2[:], in_=wb[1345:1410, :])
            wagga = pp.tile([65, 64], bf16)
            nc.sync.dma_start(out=wagga[:], in_=wb[1410:1475, :])
            waggb = pp.tile([64, 64], bf16)
            nc.sync.dma_start(out=waggb[:], in_=wb[1475:1539, :])
            wcx = pp.tile([65, 64], bf16)
            nc.sync.dma_start(out=wcx[:], in_=wb[1539:1604, :])

            ident = pp.tile([128, 128], bf16)
            make_identity(nc, ident[:])

            zrow = pp.tile([1, 64], bf16)
            nc.vector.memset(zrow[:], 0.0)
            nrow = pp.tile([1, 64], bf16)
            nc.vector.memset(nrow[:], NEGBIG)

            # ---- own-shard encoder nT (feature-major) ----
            sc_enc = nc.enter_named_scope("enc", False)
            nT = pp.tile([65, NSP], bf16)
            nc.vector.memset(nT[64:65, :], 1.0)
            with tc.tile_pool(name="psA", bufs=2, space="PSUM") as psA:
                for b in range(NSP // 512):
                    sl = slice(b * 512, (b + 1) * 512)
                    xch = wk.tile([4, 512], fp32, tag="xch")
                    nc.sync.dma_start(out=xch[:], in_=xT[:, sl])
                    p1 = psA.tile([64, 512], fp32, space="PSUM", tag="pa")
                    nc.tensor.matmul(p1[:], lhsT=wnx[:], rhs=xch[:], start=True, stop=True)
                    nc.scalar.activation(
                        nT[0:64, sl], p1[:], mybir.ActivationFunctionType.Relu
                    )

            nc.leave_named_scope("enc", sc_enc[0], False)
            # ---- layer 1: masked dense compute (j-outer, SBUF accumulator) ----
            sc_l1 = nc.enter_named_scope("L1", False)
            hT = pp.tile([65, NSP], bf16)
            nc.vector.memset(hT[64:65, :], 1.0)
            UBLK = 32  # chunks of U per staged load
            with tc.tile_pool(name="psMd", bufs=2, space="PSUM") as psMd, tc.tile_pool(
                name="psAcc", bufs=2, space="PSUM"
            ) as psAcc, tc.tile_pool(
                name="psT", bufs=2, space="PSUM"
            ) as psT, tc.tile_pool(name="uw", bufs=3) as uw, tc.tile_pool(
                name="uv", bufs=8
            ) as uv, tc.tile_pool(name="a1p", bufs=1) as a1p:
                acc1 = a1p.tile([128, NCH, 64], fp32)
                nc.vector.memset(acc1[:], 0.0)
                for jj in range(0 if SKIP_L1MD else maxd1):
                    nchj = int(C1[jj])
                    for b0 in range(0, nchj, UBLK):
                        b1 = min(nchj, b0 + UBLK)
                        ucb = (int(ucol[jj]) + b0) * 128
                        ust = uw.tile([56, UBLK * 128], bf16, tag="ust")
                        nc.sync.dma_start(
                            out=ust[:, 0 : (b1 - b0) * 128],
                            in_=U_d[:, ucb : ucb + (b1 - b0) * 128],
                        )
                        for ch in range(b0, b1):
                            us = slice((ch - b0) * 128, (ch - b0 + 1) * 128)
                            mdp1 = psMd.tile([128, 384], fp32, space="PSUM", tag="md1")
                            mdp2 = psMd.tile([128, 384], fp32, space="PSUM", tag="md2")
                            for k in range(6):
                                tgt = mdp1 if k < 3 else mdp2
                                fs = slice((k % 3) * 128, (k % 3) * 128 + 128)
                                nc.tensor.matmul(
                                    tgt[:, fs],
                                    lhsT=wqt[:, 128 * k : 128 * (k + 1)],
                                    rhs=ust[:, us],
                                    start=True,
                                    stop=True,
                                )
                            mds = uv.tile([128, 6, 128], bf16, tag="mds")
                            nc.scalar.activation(
                                mds[:, 0:3, :].rearrange("p a f -> p (a f)"),
                                mdp1[:],
                                mybir.ActivationFunctionType.Relu,
                            )
                            nc.scalar.activation(
                                mds[:, 3:6, :].rearrange("p a f -> p (a f)"),
                                mdp2[:],
                                mybir.ActivationFunctionType.Relu,
                            )
                            msg = psAcc.tile([128, 64], fp32, space="PSUM", tag="msg")
                            for k in range(6):
                                nc.tensor.matmul(
                                    msg[:],
                                    lhsT=mds[:, k, :],
                                    rhs=wstk[:, k, :],
                                    start=(k == 0),
                                    stop=(k == 5),
                                )
                            nc.vector.tensor_tensor(
                                acc1[:, ch, :], acc1[:, ch, :], msg[:], op=AX.add
                            )
                # finalize: root term + accumulated messages, relu, transpose
                for ch in range(NCH):
                    sl = slice(ch * 128, (ch + 1) * 128)
                    p = psAcc.tile([128, 64], fp32, space="PSUM", tag="msg")
                    nc.tensor.matmul(
                        p[:], lhsT=nT[:, sl], rhs=wroot1[:], start=True, stop=True
                    )
                    nc.vector.tensor_tensor(p[:], p[:], acc1[:, ch, :], op=AX.add)
                    hch = wk.tile([128, 64], bf16, tag="hch")
                    nc.vector.tensor_scalar_max(hch[:], p[:], 0.0)
                    pt = psT.tile([64, 128], bf16, space="PSUM", tag="pt")
                    nc.tensor.transpose(pt[:], hch[:], ident[:])
                    nc.scalar.activation(
                        hT[0:64, sl], pt[:], mybir.ActivationFunctionType.Copy
                    )

            nc.leave_named_scope("L1", sc_l1[0], False)
            # ---- AllGather raw hT, then build the 8-relation table locally ----
            sc_ag = nc.enter_named_scope("AG", False)
            nc.sync.dma_start(out=hT_in[:, :], in_=hT[0:64, :])
            if SIM_NO_COLL:
                for c in range(NCORES):
                    nc.sync.dma_start(
                        out=hT_all[c * 64 : (c + 1) * 64, :], in_=hT_in[:, :]
                    )
            else:
                nc.gpsimd.collective_compute(
                    "AllGather",
                    mybir.AluOpType.bypass,
                    replica_groups=groups,
                    ins=[hT_in[:, :]],
                    outs=[hT_all[:, :]],
                )
            nc.leave_named_scope("AG", sc_ag[0], False)
            sc_ht = nc.enter_named_scope("Htab", False)
            with tc.tile_pool(name="psB", bufs=2, space="PSUM") as psB, tc.tile_pool(
                name="hw", bufs=3
            ) as hw:
                for c in range(NCORES):
                    for b in range(NSP // 512):
                        hg = hw.tile([64, 512], bf16, tag="hg")
                        nc.sync.dma_start(
                            out=hg[:],
                            in_=hT_all[c * 64 : (c + 1) * 64, b * 512 : (b + 1) * 512],
                        )
                        for q in range(4):
                            ch = b * 4 + q
                            p = psB.tile([128, R * 64], fp32, space="PSUM", tag="p")
                            nc.tensor.matmul(
                                p[:],
                                lhsT=hg[:, q * 128 : (q + 1) * 128],
                                rhs=w2a[:],
                                start=True,
                                stop=True,
                            )
                            stagH = wk.tile([128, R, 64], bf16, tag="stagH")
                            nc.scalar.activation(
                                stagH[:].rearrange("p r f -> p (r f)"),
                                p[:],
                                mybir.ActivationFunctionType.Copy,
                            )
                            row0 = (c * NSP + ch * 128) * R
                            nc.sync.dma_start(
                                out=agH_out[row0 : row0 + 128 * R, :].rearrange(
                                    "(p r) f -> p (r f)", r=R
                                ),
                                in_=stagH[:].rearrange("p r f -> p (r f)"),
                            )
            nc.sync.dma_start(out=agH_out[ZROW : ZROW + 1, :], in_=zrow[:])
            nc.sync.dma_start(out=agH_out[NROW : NROW + 1, :], in_=nrow[:])

            nc.leave_named_scope("Htab", sc_ht[0], False)
            # ---- layer-2 max: per-chunk staged gathers + DVE max folds (slabs) ----
            sc_l2 = nc.enter_named_scope("L2max", False)
            with tc.tile_pool(name="l2p", bufs=1) as l2p, tc.tile_pool(
                name="gp2", bufs=4
            ) as gp2:
                a2s = l2p.tile([128, CH2, 64], bf16, tag="a2s")
                if SKIP_L2:
                    nc.vector.memset(a2s[:], 0.0)
                qn = 0
                for j in range(0 if SKIP_L2 else maxd2):
                    c0 = int(col_l2[j])
                    hi = int(C2[j])
                    for b0 in range(0, hi, KB):
                        b1 = min(hi, b0 + KB)
                        if j == 0:
                            # pass 0 covers every slot: per-chunk gathers in
                            # place, no memset / staging / max-fold needed
                            for ch in range(b0, b1):
                                qn += 1
                                gih = nc.gpsimd.indirect_dma_start(
                                    out=a2s[:, ch, :],
                                    out_offset=None,
                                    in_=agH_out[:, :],
                                    in_offset=bass.IndirectOffsetOnAxis(
                                        ap=idxs[:, c0 + ch : c0 + ch + 1], axis=0
                                    ),
                                )
                                gih.ins.queue = f"qPoolDynamic{qn % 4 or ''}"
                            continue
                        # gather the whole batch into one staging tile
                        # (disjoint slices -> parallel), then one wide fold
                        stg = gp2.tile([128, KB, 64], bf16, tag="stg")
                        for ch in range(b0, b1):
                            qn += 1
                            gih = nc.gpsimd.indirect_dma_start(
                                out=stg[:, ch - b0, :],
                                out_offset=None,
                                in_=agH_out[:, :],
                                in_offset=bass.IndirectOffsetOnAxis(
                                    ap=idxs[:, c0 + ch : c0 + ch + 1], axis=0
                                ),
                            )
                            gih.ins.queue = f"qPoolDynamic{qn % 4 or ''}"
                        nc.vector.tensor_tensor(
                            a2s[:, b0:b1, :],
                            a2s[:, b0:b1, :],
                            stg[:, 0 : b1 - b0, :],
                            op=AX.max,
                        )
                nc.sync.dma_start(
                    out=agH_out[QOFF : QOFF + CH2 * 128, :].rearrange(
                        "(ch p) f -> p ch f", p=128
                    ),
                    in_=a2s[:, :, :],
                )
            nc.sync.dma_start(
                out=agH_out[QOFF + CH2 * 128 : QOFF + CH2 * 128 + 1, :], in_=zrow[:]
            )

            nc.leave_named_scope("L2max", sc_l2[0], False)
            # ---- stage2: per-node sum of its nonempty-rel maxes ----
            sc_s2 = nc.enter_named_scope("S2", False)
            acc2e = pp.tile([128, NCH, 64], bf16)
            nc.vector.memset(acc2e[:], 0.0)
            with tc.tile_pool(name="gp3", bufs=4) as gp3:
                qn = 0
                for k in range(0 if SKIP_S2 else 8):
                    c0 = int(col_s2[k])
                    hi = int(CS2[k])
                    for b0 in range(0, hi, KB):
                        b1 = min(hi, b0 + KB)
                        if k == 0:
                            # k=0 covers every node (zero row for edgeless
                            # ones): per-chunk gathers in place over memset
                            for ch in range(b0, b1):
                                qn += 1
                                gih = nc.gpsimd.indirect_dma_start(
                                    out=acc2e[:, ch, :],
                                    out_offset=None,
                                    in_=agH_out[:, :],
                                    in_offset=bass.IndirectOffsetOnAxis(
                                        ap=idxs[:, c0 + ch : c0 + ch + 1], axis=0
                                    ),
                                )
                                gih.ins.queue = f"qPoolDynamic{qn % 4 or ''}"
                            continue
                        stg = gp3.tile([128, KB, 64], bf16, tag="stg")
                        for ch in range(b0, b1):
                            qn += 1
                            gih = nc.gpsimd.indirect_dma_start(
                                out=stg[:, ch - b0, :],
                                out_offset=None,
                                in_=agH_out[:, :],
                                in_offset=bass.IndirectOffsetOnAxis(
                                    ap=idxs[:, c0 + ch : c0 + ch + 1], axis=0
                                ),
                            )
                            gih.ins.queue = f"qPoolDynamic{qn % 4 or ''}"
                        nc.vector.tensor_tensor(
                            acc2e[:, b0:b1, :],
                            acc2e[:, b0:b1, :],
                            stg[:, 0 : b1 - b0, :],
                            op=AX.add,
                        )

            nc.leave_named_scope("S2", sc_s2[0], False)
            # ---- h2 = relu(root2 + acc2e); head ----
            sc_hd = nc.enter_named_scope("head", False)
            y = pp.tile([128, NCH], fp32)
            psC_cm = tc.tile_pool(name="psC", bufs=4, space="PSUM")
            psC = psC_cm.__enter__()
            for ch in range(NCH):
                sl = slice(ch * 128, (ch + 1) * 128)
                p = psC.tile([128, 64], fp32, space="PSUM", tag="pc")
                nc.tensor.matmul(
                    p[:], lhsT=hT[:, sl], rhs=wroot2[:], start=True, stop=True
                )
                nc.vector.tensor_tensor(p[:], p[:], acc2e[:, ch, :], op=AX.add)
                h2 = wk.tile([128, 64], bf16, tag="h2")
                nc.vector.tensor_scalar_max(h2[:], p[:], 0.0)
                pt = psC.tile([64, 128], bf16, space="PSUM", tag="pc")
                nc.tensor.transpose(pt[:], h2[:], ident[:])
                h2T = wk.tile([65, 128], bf16, tag="h2T")
                nc.scalar.activation(
                    h2T[0:64, :], pt[:], mybir.ActivationFunctionType.Copy
                )
                nc.vector.memset(h2T[64:65, :], 1.0)
                if ch % 4 == 0:
                    sl4 = slice(ch * 128, min((ch + 4) * 128, NSP))
                    omch = wk.tile([3, 512], fp32, tag="omch")
                    nc.sync.dma_start(out=omch[:, 0 : sl4.stop - sl4.start], in_=omT[:, sl4])
                    po4 = psC.tile([64, 512], fp32, space="PSUM", tag="po4")
                    nc.tensor.matmul(
                        po4[:], lhsT=wox[:], rhs=omch[:], start=True, stop=True
                    )
                    oT4 = wk.tile([64, 512], bf16, tag="oT4")
                    nc.scalar.activation(
                        oT4[:], po4[:], mybir.ActivationFunctionType.Relu
                    )
                oTc = oT4[:, (ch % 4) * 128 : (ch % 4 + 1) * 128]
                p3 = psC.tile([128, 64], fp32, space="PSUM", tag="pc")
                nc.tensor.matmul(p3[:], lhsT=h2T[:], rhs=wagga[:], start=True, stop=False)
                nc.tensor.matmul(
                    p3[:], lhsT=oTc, rhs=waggb[:], start=False, stop=True
                )
                h3 = wk.tile([128, 64], bf16, tag="h3")
                nc.vector.tensor_scalar_max(h3[:], p3[:], 0.0)
                pt2 = psC.tile([64, 128], bf16, space="PSUM", tag="pc")
                nc.tensor.transpose(pt2[:], h3[:], ident[:])
                h3T = wk.tile([65, 128], bf16, tag="h3T")
                nc.scalar.activation(
                    h3T[0:64, :], pt2[:], mybir.ActivationFunctionType.Copy
                )
                nc.vector.memset(h3T[64:65, :], 1.0)
                py = psC.tile([128, 64], fp32, space="PSUM", tag="pc")
                nc.tensor.matmul(py[:], lhsT=h3T[:], rhs=wcx[:], start=True, stop=True)
                nc.scalar.activation(
                    y[:, ch : ch + 1],
                    py[:, 0:1],
                    mybir.ActivationFunctionType.Tanh,
                )
            nc.vector.tensor_scalar_mul(y[:], y[:], 5.0)
            nc.sync.dma_start(out=out_d[:, :], in_=y[:])
            psC_cm.__exit__(None, None, None)
            nc.leave_named_scope("head", sc_hd[0], False)

    nc.compile()
    return nc


# ---------------------------------------------------------------------------
# entry point
# ---------------------------------------------------------------------------

_CACHE = {}
LAST_RUN_SECONDS = None


def kernel(**inputs):
    import time
    from concourse.bass_utils import run_bass_kernel_spmd

    global LAST_RUN_SECONDS
    edge_index = np.asarray(inputs["edge_index"])
    edge_type = np.asarray(inputs["edge_type"])

    import hashlib

    key = hashlib.md5(edge_index.tobytes() + edge_type.tobytes()).hexdigest()
    if key not in _CACHE:
        cores, common = preprocess(edge_index, edge_type)
        nc = build_graph(common)
        _CACHE[key] = (cores, common, nc)
    cores, common, nc = _CACHE[key]
    in_maps = build_core_inputs(inputs, cores, common)

    t0 = time.time()
    res = run_bass_kernel_spmd(nc, in_maps, core_ids=list(range(NCORES)))
    LAST_RUN_SECONDS = time.time() - t0

    out = np.empty((N, 1), np.float32)
    for c in range(NCORES):
        o = res.results[c]["out"]  # [128, NCH]
        ranks = cores[c]["rank"]
        out[c * NS : (c + 1) * NS, 0] = o[ranks % 128, ranks // 128]
    return out


if __name__ == "__main__":
    import reference

    inputs = reference.setup_inputs()
    expected = np.asarray(reference.reference(**inputs))
    got = kernel(**{k: np.asarray(v) for k, v in inputs.items()})
    rel = np.linalg.norm(got - expected) / np.linalg.norm(expected)
    print(f"Relative error: {rel:.3e}")

